# revision 46
# baseline (speedup 1.0000x reference)
"""TRN2 Bass kernel for nn_MultiHeadAttention (GQA + RoPE + causal, dense transformer).

Sharding: tensor-parallel over kv-head groups (TP=4; each core owns 2 kv heads
plus their 8 grouped q heads) x data-parallel over batch (DP=2) -> 8 cores.
The host sums the 4 partial o-projection outputs per batch element (the TP
all-reduce) and transposes back.

Per-core schedule (windowed software pipeline over 4 t-blocks of 512 q rows):
  W(-1): weight/const DMAs + chunk-0 projections.
  W(t):  attention for t-block t (scores -> exp -> PV, with per-pair softmax
         normalize folded in) interleaved at u-step granularity with chunk
         t+1 projections and the o-projection of t-block t-1, so the PE queue
         always has independent matmuls to chew while the scalar engine exps.

Key layout tricks vs the straightforward version:
  - Head pairing (m, m+4): q-proj m-tile stacks a kv0-group head (rows 0-63)
    with a kv1-group head (rows 64-127); score matmuls use half-zero weights
    ZkT (K_kv0 top) / ZkB (K_kv1 bottom) so no q duplication is needed.
  - RoPE head-dim layout [e0-15|o0-15|e16-31|o16-31]: the rotate-half partner
    is p XOR 16, an intra-quadrant move done by one DVE stream_shuffle
    (no SBUF-SBUF DMAs on the scalar queue).
  - V transposes via the DMA XBAR (dma_start_transpose) straight into vext,
    with host-interleaved V columns to match the 3D-output mapping.
  - Softmax denominators ride the PV matmul as a 65th ones-column; normalize
    uses reciprocal_approx_fast in place + partition_broadcast from the
    denominator row (no DMA round-trips).
  - All HBM loads are host-pre-tiled to [128, *] contiguous layouts.

Numerics: bf16 matmul paths with fp32 PSUM accumulation; softmax denominators
and normalization in f32.
"""
from collections import deque
from contextlib import ExitStack

import numpy as np
import ml_dtypes

import concourse.bass as bass
import concourse.mybir as mybir
import concourse.tile as tile
from concourse import bacc
from concourse.bass_utils import run_bass_kernel_spmd
from concourse.masks import make_identity

F32 = mybir.dt.float32
BF16 = mybir.dt.bfloat16
DT = BF16
AF = mybir.ActivationFunctionType

N_CORES = 8
B, S, D = 2, 2048, 2048
HQ_TOT, HKV_TOT, HD = 32, 8, 64
ROPE_BASE = 10000.0
TP = N_CORES // B          # 4 cores per batch element
HQ = HQ_TOT // TP          # 8 q heads per core
HKV = HKV_TOT // TP        # 2 kv heads per core
DK = D // 128              # contraction k-tiles (16)
T = S // 512               # t-blocks of 512 q rows (4)
KT = S // 128              # kpos tiles of 128 (16)
CH = 512                   # position-chunk width
QM = 4                     # q-proj m-tiles (head pairs)
SCALE = float(HD) ** -0.5
SHUF_ROPE = [i ^ 16 for i in range(32)]


def build(nc):
    hst = nc.dram_tensor("hst", [128, T * DK * CH], DT, kind="ExternalInput")
    wqt = nc.dram_tensor("wqt", [128, DK * 512], DT, kind="ExternalInput")
    wkvt = nc.dram_tensor("wkvt", [128, DK * 256], DT, kind="ExternalInput")
    wot = nc.dram_tensor("wot", [128, 4 * D], DT, kind="ExternalInput")
    cosd = nc.dram_tensor("cosd", [128, S], DT, kind="ExternalInput")
    sind = nc.dram_tensor("sind", [128, S], DT, kind="ExternalInput")
    lmd = nc.dram_tensor("lmd", [128, KT], F32, kind="ExternalInput")
    outt = nc.dram_tensor("outt", [128, 16 * T * 512], DT, kind="ExternalOutput")

    ctx = ExitStack()
    with tile.TileContext(nc) as tc:
        consts = ctx.enter_context(tc.tile_pool(name="consts", bufs=1))
        big = ctx.enter_context(tc.tile_pool(name="big", bufs=1))
        hsp = ctx.enter_context(tc.tile_pool(name="hsp", bufs=3))
        wp = ctx.enter_context(tc.tile_pool(name="wp", bufs=2))
        qrp = ctx.enter_context(tc.tile_pool(name="qrp", bufs=8))
        pp = ctx.enter_context(tc.tile_pool(name="pp", bufs=2, space="PSUM"))

        # ---------------- lead-in DMAs ----------------
        # wkvt host layout: [K cols (DK*128) | V cols (DK*128)];
        # wqt host layout: m-tile major [(m, k, 128)].
        # Load in first-use order (K, hs0, q0-cols, V, q1-3-cols) so the
        # first projection matmuls start as soon as their k-tiles land.
        wkv_sb = consts.tile([128, DK * 256], DT, name="wkvsb")
        wq_sb = consts.tile([128, DK * 512], DT, name="wqsb")

        hs_tiles = {}

        def load_hs(c, quarters=range(4)):
            # a DMA queue serializes its transfers at ~100GB/s: split the
            # chunk across the sync and gpsimd queues.
            if c in hs_tiles:
                hg = hs_tiles[c]
            else:
                hg = hsp.tile([128, DK * CH], DT, tag="hs", name=f"hs{c}")
                hs_tiles[c] = hg
            for q in quarters:
                eng = nc.sync if q % 2 == 0 else nc.gpsimd
                eng.dma_start(
                    hg[:, q * 2048:(q + 1) * 2048],
                    hst[:, c * DK * CH + q * 2048: c * DK * CH + (q + 1) * 2048])

        # lead-in spread over the three DMA-capable queues in first-use order
        for q in range(4):
            nc.scalar.dma_start(wkv_sb[:, q * 512:(q + 1) * 512],
                                wkvt[:, q * 512:(q + 1) * 512])
            load_hs(0, [q])
        nc.scalar.dma_start(wq_sb[:, 0:2048], wqt[:, 0:2048])
        cos_sb = consts.tile([128, S], DT, name="cossb")
        nc.sync.dma_start(cos_sb[:], cosd[:, :])
        sin_sb = consts.tile([128, S], DT, name="sinsb")
        nc.gpsimd.dma_start(sin_sb[:], sind[:, :])
        nc.scalar.dma_start(wkv_sb[:, 2048:4096], wkvt[:, 2048:4096])
        for m in range(1, QM):
            nc.scalar.dma_start(wq_sb[:, m * 2048:(m + 1) * 2048],
                                wqt[:, m * 2048:(m + 1) * 2048])
        lm_sb = consts.tile([128, KT], F32, name="lmsb")
        nc.sync.dma_start(lm_sb[:], lmd[:, :])
        wo_sb = consts.tile([128, 4 * D], DT, name="wosb")

        # ---------------- resident tensors ----------------
        ZkT = big.tile([128, S], DT, name="ZkT")
        ZkB = big.tile([128, S], DT, name="ZkB")
        nc.gpsimd.memset(ZkT[:], 0.0)
        nc.gpsimd.memset(ZkB[:], 0.0)
        vext = [big.tile([128, 2 * 65], DT, name=f"vext{u}") for u in range(KT)]
        for u in range(KT):
            nc.vector.memset(vext[u][:, 64:65], 1.0)
            nc.vector.memset(vext[u][:, 129:130], 1.0)
        attnT = [big.tile([128, S], DT, name=f"attnT{k}") for k in range(QM)]
        ident = consts.tile([128, 128], DT, name="ident")
        make_identity(nc, ident[:])
        ones64 = consts.tile([1, 64], F32, name="ones64")
        nc.vector.memset(ones64[:], 1.0)
        tri01 = consts.tile([128, 128], DT, name="tri01")
        nc.vector.memset(tri01[:], 1.0)
        nc.gpsimd.affine_select(
            out=tri01[:], in_=tri01[:], compare_op=mybir.AluOpType.is_ge,
            fill=0.0, base=0, channel_multiplier=-1, pattern=[[1, 128]])
        # prime the ACT exp table while the PE is still in lead-in
        warm = wp.tile([1, 32], F32, tag="warm")
        nc.vector.memset(warm[:], 0.0)
        nc.scalar.activation(warm[:], warm[:], AF.Exp)

        qr_tiles = {}

        # ---------------- rope ----------------
        def rope_into(dst, src, col0):
            P = wp.tile([128, CH], DT, tag="ropeP")
            nc.vector.stream_shuffle(P[:], src[:], mask=SHUF_ROPE)
            m1 = wp.tile([128, CH], DT, tag="ropem1")
            nc.vector.tensor_mul(m1[:], src[:], cos_sb[:, col0:col0 + CH])
            m2 = wp.tile([128, CH], DT, tag="ropem2")
            nc.vector.tensor_mul(m2[:], P[:], sin_sb[:, col0:col0 + CH])
            nc.vector.tensor_add(dst[:], m1[:], m2[:])

        # ---------------- per-chunk projection units ----------------
        def proj_mms(pj, wsl_fn, hs):
            for k in range(DK):
                nc.tensor.matmul(pj[:], wsl_fn(k), hs[:, k * CH:(k + 1) * CH],
                                 start=(k == 0), stop=(k == DK - 1))

        def unit_k(c):
            def emit():
                col0 = c * CH
                pj = pp.tile([128, CH], F32, tag="pj", name=f"pk{c}")
                proj_mms(pj, lambda k: wkv_sb[:, k * 128:(k + 1) * 128],
                         hs_tiles[c])
                kc = wp.tile([128, CH], DT, tag="kc", name=f"kc{c}")
                nc.vector.tensor_copy(kc[:], pj[:])
                kr = wp.tile([128, CH], DT, tag="kr", name=f"kr{c}")
                rope_into(kr, kc, col0)
                nc.vector.tensor_copy(ZkT[0:64, col0:col0 + CH], kr[0:64, :])
                nc.vector.tensor_copy(ZkB[64:128, col0:col0 + CH], kr[64:128, :])
            return emit

        def unit_v(c):
            def emit():
                pj = pp.tile([128, CH], F32, tag="pj", name=f"pv{c}")
                proj_mms(pj, lambda k: wkv_sb[:, 2048 + k * 128:
                                              2048 + (k + 1) * 128],
                         hs_tiles[c])
                vc = wp.tile([128, CH], DT, tag="vc", name=f"vc{c}")
                nc.vector.tensor_copy(vc[:], pj[:])
                for uu in range(4):
                    u = 4 * c + uu
                    ps_t = pp.tile([128, 128], DT, tag="pj", name=f"pt{u}")
                    nc.tensor.transpose(ps_t[:], vc[:, uu * 128:(uu + 1) * 128],
                                        ident[:])
                    dst = vext[u][:].rearrange("p (j cc) -> p j cc", j=2)[:, :, 0:64]
                    nc.vector.tensor_copy(
                        dst, ps_t[:].rearrange("p (j cc) -> p j cc", j=2))
            return emit

        def unit_q(c, m):
            def emit():
                col0 = c * CH
                pj = pp.tile([128, CH], F32, tag="pj", name=f"pq{c}_{m}")
                proj_mms(pj, lambda k: wq_sb[:, m * 2048 + k * 128:
                                             m * 2048 + (k + 1) * 128],
                         hs_tiles[c])
                qc = wp.tile([128, CH], DT, tag="qc", name=f"qc{c}_{m}")
                nc.vector.tensor_copy(qc[:], pj[:])
                qr = qrp.tile([128, CH], DT, tag="qr", name=f"qr{c}_{m}")
                rope_into(qr, qc, col0)
                qr_tiles[(c, m)] = qr
            return emit

        def chunk_units(c):
            return ([unit_k(c), unit_v(c)] + [unit_q(c, m) for m in range(QM)])

        # ---------------- o-projection units ----------------
        def _osb_out(t, mD, po):
            osb = wp.tile([128, 512], DT, tag="osb", bufs=4)
            if t == T - 1:
                # tail: both engines idle; split-copy halves the latency
                nc.scalar.copy(osb[:, 0:256], po[:, 0:256])
                nc.vector.tensor_copy(osb[:, 256:512], po[:, 256:512])
            elif mD % 2 and t != 2:
                # C(2) runs inside the ACT-tight window 3: keep its copies
                # off the scalar queue there.
                nc.scalar.copy(osb[:], po[:])
            else:
                nc.vector.tensor_copy(osb[:], po[:])
            oeng = nc.sync if mD % 2 == 0 else nc.gpsimd
            oeng.dma_start(
                outt[:, (mD * T + t) * 512:(mD * T + t) * 512 + 512],
                osb[:])

        def unit_oproj(t, mD):
            def emit():
                po = pp.tile([128, 512], F32, tag="pj", name=f"po{t}_{mD}")
                for k in range(QM):
                    nc.tensor.matmul(
                        po[:],
                        wo_sb[:, k * D + mD * 128:k * D + (mD + 1) * 128],
                        attnT[k][:, t * 512:(t + 1) * 512],
                        start=(k == 0), stop=(k == QM - 1))
                _osb_out(t, mD, po)
            return emit

        # split oproj for the tail: accumulate k=0..2 (pairs already normed)
        # while the last pair's normalize chain runs, finish k=3 after.
        po_pre = {}

        def oproj_pre(t, mD):
            def emit():
                po = pp.tile([128, 512], F32, tag="pj", name=f"poP{t}_{mD}")
                for k in range(QM - 1):
                    nc.tensor.matmul(
                        po[:],
                        wo_sb[:, k * D + mD * 128:k * D + (mD + 1) * 128],
                        attnT[k][:, t * 512:(t + 1) * 512],
                        start=(k == 0), stop=False)
                po_pre[(t, mD)] = po
            return emit

        def oproj_fin(t, mD):
            def emit():
                po = po_pre.pop((t, mD))
                k = QM - 1
                nc.tensor.matmul(
                    po[:],
                    wo_sb[:, k * D + mD * 128:k * D + (mD + 1) * 128],
                    attnT[k][:, t * 512:(t + 1) * 512],
                    start=False, stop=True)
                _osb_out(t, mD, po)
            return emit

        # ---------------- attention steps for one t-block ----------------
        def a_steps(t):
            pairs = []
            nu = 4 * t + 4
            for mp in range(QM):
                steps = []
                pairs.append(steps)
                st = {}
                pa = [None, None]

                def mk_pa(mp=mp, pa=pa):
                    def f():
                        pa[0] = pp.tile([65, 512], F32, tag="pa",
                                        name=f"pa0_{t}_{mp}")
                        pa[1] = pp.tile([65, 512], F32, tag="pa",
                                        name=f"pa1_{t}_{mp}")
                    return f

                def mk_S(u, mp=mp, st=st):
                    def f():
                        w = u - 4 * t
                        c0 = 0 if w < 0 else 128 * w
                        qr = qr_tiles[(t, mp)]
                        s2 = pp.tile([128, 1024], F32, tag="s2",
                                     name=f"s2_{t}_{mp}_{u}")
                        nc.tensor.matmul(s2[:, c0:512],
                                         ZkT[:, u * 128:(u + 1) * 128],
                                         qr[:, c0:512], start=True, stop=True)
                        nc.tensor.matmul(s2[:, 512 + c0:1024],
                                         ZkB[:, u * 128:(u + 1) * 128],
                                         qr[:, c0:512], start=True, stop=True)
                        st[u] = s2
                    return f

                def mk_EP(u, mp=mp, st=st, pa=pa):
                    def f():
                        w = u - 4 * t
                        c0 = 0 if w < 0 else 128 * w
                        s2 = st.pop(u)
                        pr2 = wp.tile([128, 1024], DT, tag="pr", bufs=4,
                                      name=f"pr{t}_{mp}_{u}")
                        if c0 == 0:
                            nc.scalar.activation(pr2[:], s2[:], AF.Exp,
                                                 bias=lm_sb[:, u:u + 1],
                                                 scale=SCALE)
                        else:
                            view = lambda x: x[:].rearrange(
                                "p (h q) -> p h q", h=2)[:, :, c0:512]
                            nc.scalar.activation(view(pr2), view(s2), AF.Exp,
                                                 bias=lm_sb[:, u:u + 1],
                                                 scale=SCALE)
                        if w >= 0:
                            for hh in range(2):
                                base = hh * 512 + c0
                                nc.vector.tensor_mul(pr2[:, base:base + 128],
                                                     pr2[:, base:base + 128],
                                                     tri01[:])
                        for hh in range(2):
                            nc.tensor.matmul(
                                pa[hh][:, c0:512],
                                vext[u][:, hh * 65:(hh + 1) * 65],
                                pr2[:, hh * 512 + c0:(hh + 1) * 512],
                                start=(u == 0), stop=(u == nu - 1))
                    return f

                def mk_norm(mp=mp, pa=pa):
                    def f():
                        # normalize: denominator row hops to partition 0 via
                        # a DVE shifted copy (gpsimd/custom-DVE ops mishandle
                        # partition-offset views on HW, plain DVE copies
                        # don't), then recip -> broadcast -> scale.
                        tail = (t == T - 1 and mp == QM - 1)
                        for hh in range(2):
                            rcp = wp.tile([1, 512], F32, tag="rcp", bufs=4,
                                          name=f"rcp{hh}_{t}_{mp}")
                            nc.vector.tensor_copy(rcp[0:1, :],
                                                  pa[hh][64:65, :])
                            nc.vector.reciprocal_approx_fast(
                                rcp[0:1, :], rcp[0:1, :])
                            a_sb = wp.tile([64, 512], F32, tag="asb", bufs=4,
                                           name=f"a{hh}_{t}_{mp}")
                            if tail:
                                nc.scalar.copy(a_sb[:], pa[hh][0:64, :])
                            else:
                                nc.vector.tensor_copy(a_sb[:], pa[hh][0:64, :])
                            if tail:
                                # PE outer-product broadcast keeps the array
                                # busy (HAM warm) through the tail chain
                                bb = pp.tile([64, 512], F32, tag="s2",
                                             name=f"bbp{hh}_{t}_{mp}")
                                nc.tensor.matmul(bb[:], ones64[0:1, :],
                                                 rcp[0:1, :],
                                                 start=True, stop=True)
                            else:
                                bb = wp.tile([64, 512], F32, tag="bb")
                                nc.gpsimd.partition_broadcast(bb[:],
                                                              rcp[0:1, :])
                            nc.vector.tensor_mul(
                                attnT[mp][hh * 64:hh * 64 + 64,
                                          t * 512:(t + 1) * 512],
                                a_sb[0:64, :], bb[:])
                    return f

                def fuse(*fs):
                    def f():
                        for g in fs:
                            g()
                    return f

                steps.append(fuse(mk_pa(), mk_S(0)))
                steps.append(mk_S(1))
                for u in range(2, nu):
                    steps.append(fuse(mk_EP(u - 2), mk_S(u)))
                steps.append(mk_EP(nu - 2))
                if t == T - 1 and mp == QM - 1:
                    steps.append(fuse(mk_EP(nu - 1), oproj_pre(t, 0),
                                      oproj_pre(t, 1), mk_norm()))
                else:
                    steps.append(fuse(mk_EP(nu - 1), mk_norm()))
            return pairs

        # ---------------- windowed schedule ----------------
        def run_window(t, pre=()):
            fillers = deque()
            # build filler list: chunk t+1 projections and oproj of t-1,
            # interleaved so pj-tag allocations alternate users
            proj = list(pre) + (chunk_units(t + 1) if t + 1 < T else [])
            opro = [unit_oproj(t - 1, mD) for mD in range(16)] if t >= 1 else []
            # merge proportionally
            na, nb = len(proj), len(opro)
            ia = ib = 0
            while ia < na or ib < nb:
                if ib * max(na, 1) <= ia * max(nb, 1) and ib < nb:
                    fillers.append(opro[ib]); ib += 1
                elif ia < na:
                    fillers.append(proj[ia]); ia += 1
                else:
                    fillers.append(opro[ib]); ib += 1
            pairs = a_steps(t)
            if t + 2 < T:
                load_hs(t + 2)
            if t == T - 1:
                # drain all fillers before the last pair so its fused oproj
                # prologue owns the pj PSUM slots (and PE stays warm through
                # the final normalize chain)
                head = [s for pr in pairs[:-1] for s in pr]
                tail_steps = pairs[-1]
            else:
                head = [s for pr in pairs for s in pr]
                tail_steps = []
            ratio = len(fillers) / max(len(head), 1)
            acc = 0.0
            for stp in head:
                stp()
                acc += ratio
                while acc >= 1.0 and fillers:
                    fillers.popleft()()
                    acc -= 1.0
            while fillers:
                fillers.popleft()()
            for stp in tail_steps:
                stp()

        # lead-in: only the chunk-0 units A(0)'s first head-pair needs;
        # the remaining q-projections and the hs1/wo loads become window-0
        # fillers so attention matmuls start while lead-in DMAs drain.
        unit_k(0)()
        unit_q(0, 0)()
        unit_v(0)()
        pre0 = ([lambda: load_hs(1)]
                + [unit_q(0, m) for m in range(1, QM)]
                + [lambda: nc.gpsimd.dma_start(wo_sb[:], wot[:, :])])
        run_window(0, pre=pre0)
        for t in range(1, T):
            run_window(t)
        oproj_fin(T - 1, 0)()
        oproj_fin(T - 1, 1)()
        for mD in range(2, 16):
            unit_oproj(T - 1, mD)()
        import os
        if os.environ.get("DBG_DUMP"):
            dzt = nc.dram_tensor("dbg_zkt", [128, S], DT, kind="ExternalOutput")
            dzb = nc.dram_tensor("dbg_zkb", [128, S], DT, kind="ExternalOutput")
            nc.sync.dma_start(dzt[:, :], ZkT[:])
            nc.sync.dma_start(dzb[:, :], ZkB[:])
            dve = nc.dram_tensor("dbg_vext", [128, KT * 130], DT,
                                 kind="ExternalOutput")
            for u in range(KT):
                nc.sync.dma_start(dve[:, u * 130:(u + 1) * 130], vext[u][:])
            dat = nc.dram_tensor("dbg_attnT", [128, QM * S], DT,
                                 kind="ExternalOutput")
            for k in range(QM):
                nc.sync.dma_start(dat[:, k * S:(k + 1) * S], attnT[k][:])
        ctx.close()
    return nc


def _host_prep(hidden_states, attention_mask, Wq, Wk, Wv, Wo):
    bf16 = ml_dtypes.bfloat16
    hs = np.asarray(hidden_states, np.float32)
    am = np.asarray(attention_mask)
    Wq = np.asarray(Wq, np.float32)
    Wk = np.asarray(Wk, np.float32)
    Wv = np.asarray(Wv, np.float32)
    Wo = np.asarray(Wo, np.float32)

    # rope tables in the 16-interleaved head-dim layout
    inv = 1.0 / (ROPE_BASE ** (np.arange(0, HD, 2, dtype=np.float64) / HD))
    freqs = np.arange(S, dtype=np.float64)[:, None] * inv[None, :]  # [S, 32]
    cosf = np.cos(freqs)
    sinf = np.sin(freqs)
    fidx = np.array([r % 16 + 16 * (r // 32) for r in range(64)])
    sign = np.array([-1.0 if (r // 16) % 2 == 0 else 1.0 for r in range(64)])
    cos64 = cosf[:, fidx].T                      # [64, S]
    sin64 = (sinf[:, fidx] * sign[None, :]).T    # [64, S]
    cos128 = np.concatenate([cos64, cos64], 0).astype(np.float32)
    sin128 = np.concatenate([sin64, sin64], 0).astype(np.float32)

    def perm16(w):  # [D, 64] head-dim cols -> 16-interleaved e/o layout
        return np.concatenate(
            [w[:, 0:32:2], w[:, 1:32:2], w[:, 32:64:2], w[:, 33:64:2]], axis=1)

    def tile_rows(w, ncols):  # [DK*128, ncols] -> [128, DK*ncols]
        return np.ascontiguousarray(
            w.reshape(DK, 128, ncols).transpose(1, 0, 2).reshape(128, DK * ncols))

    hst_b = []
    for b in range(B):
        hb = hs[b].reshape(T, CH, DK, 128).transpose(3, 0, 2, 1)
        hst_b.append(np.ascontiguousarray(hb.reshape(128, T * DK * CH)).astype(bf16))
    lm_b = []
    for b in range(B):
        lm = np.where(am[b] > 0, 0.0, -1e30).astype(np.float32)
        lm_b.append(np.ascontiguousarray(lm.reshape(KT, 128).T))

    in_maps = []
    for core in range(N_CORES):
        b, g = core // TP, core % TP
        h0 = g * HQ
        kv0 = g * HKV
        # q: m-tile mp = [head h0+mp | head h0+mp+4], both perm16'd
        wq_c = np.concatenate(
            [np.concatenate([perm16(Wq[:, (h0 + mp) * HD:(h0 + mp + 1) * HD]),
                             perm16(Wq[:, (h0 + mp + 4) * HD:(h0 + mp + 5) * HD])],
                            axis=1)
             for mp in range(QM)], axis=1)                      # [D, 512]
        wk_c = np.concatenate(
            [perm16(Wk[:, (kv0 + j) * HD:(kv0 + j + 1) * HD]) for j in range(2)],
            axis=1)                                             # [D, 128]
        wv_c = np.concatenate(
            [Wv[:, (kv0 + j) * HD:(kv0 + j + 1) * HD] for j in range(2)], axis=1)
        wkv_c = np.concatenate([wk_c, wv_c], axis=1)            # [D, 256]
        # o: row-block k = [head h0+k | head h0+k+4]
        wo_c = np.concatenate(
            [np.concatenate([Wo[(h0 + k) * HD:(h0 + k + 1) * HD, :],
                             Wo[(h0 + k + 4) * HD:(h0 + k + 5) * HD, :]], axis=0)
             for k in range(QM)], axis=0)                       # [512, D]
        wqt = np.concatenate(
            [tile_rows(np.ascontiguousarray(wq_c[:, m * 128:(m + 1) * 128]), 128)
             for m in range(QM)], axis=1)
        wkvt = np.concatenate(
            [tile_rows(wk_c, 128), tile_rows(np.ascontiguousarray(wv_c), 128)],
            axis=1)
        in_maps.append({
            "hst": hst_b[b],
            "wqt": np.ascontiguousarray(wqt).astype(bf16),
            "wkvt": np.ascontiguousarray(wkvt).astype(bf16),
            "wot": np.ascontiguousarray(
                wo_c.reshape(QM, 128, D).transpose(1, 0, 2)
                .reshape(128, QM * D)).astype(bf16),
            "cosd": cos128.astype(bf16),
            "sind": sin128.astype(bf16),
            "lmd": lm_b[b],
        })
    return in_maps


_NC_CACHE = {}


def _get_nc():
    if "nc" not in _NC_CACHE:
        nc = bacc.Bacc("TRN2", target_bir_lowering=False, num_devices=N_CORES)
        build(nc)
        nc.compile()
        _NC_CACHE["nc"] = nc
    return _NC_CACHE["nc"]


def kernel(hidden_states, attention_mask, Wq, Wk, Wv, Wo):
    nc = _get_nc()
    in_maps = _host_prep(hidden_states, attention_mask, Wq, Wk, Wv, Wo)
    res = run_bass_kernel_spmd(nc, in_maps, list(range(N_CORES)))
    out = np.zeros((B, S, D), np.float32)
    for core, r in enumerate(res.results):
        o = r["outt"].reshape(128, 16, T, 512).transpose(1, 0, 2, 3)
        out[core // TP] += o.reshape(D, S).T
    return out


# revision 47
# speedup vs baseline: 1.1620x; 1.1620x over previous
"""TRN2 Bass kernel for nn_MultiHeadAttention (GQA + RoPE + causal, dense transformer).

Sharding: tensor-parallel over kv-head groups (TP=4; each core owns 2 kv heads
plus their 8 grouped q heads) x data-parallel over batch (DP=2) -> 8 cores.
The host sums the 4 partial o-projection outputs per batch element (the TP
all-reduce) and transposes back.

Per-core schedule (windowed software pipeline over 4 t-blocks of 512 q rows):
  W(-1): weight/const DMAs + chunk-0 projections.
  W(t):  attention for t-block t (scores -> exp -> PV, with per-pair softmax
         normalize folded in) interleaved at u-step granularity with chunk
         t+1 projections and the o-projection of t-block t-1, so the PE queue
         always has independent matmuls to chew while the scalar engine exps.

Key layout tricks vs the straightforward version:
  - Head pairing (m, m+4): q-proj m-tile stacks a kv0-group head (rows 0-63)
    with a kv1-group head (rows 64-127); score matmuls use half-zero weights
    ZkT (K_kv0 top) / ZkB (K_kv1 bottom) so no q duplication is needed.
  - RoPE head-dim layout [e0-15|o0-15|e16-31|o16-31]: the rotate-half partner
    is p XOR 16, an intra-quadrant move done by one DVE stream_shuffle
    (no SBUF-SBUF DMAs on the scalar queue).
  - V transposes via the DMA XBAR (dma_start_transpose) straight into vext,
    with host-interleaved V columns to match the 3D-output mapping.
  - Softmax denominators ride the PV matmul as a 65th ones-column; normalize
    uses reciprocal_approx_fast in place + partition_broadcast from the
    denominator row (no DMA round-trips).
  - All HBM loads are host-pre-tiled to [128, *] contiguous layouts.

Numerics: bf16 matmul paths with fp32 PSUM accumulation; softmax denominators
and normalization in f32.
"""
from collections import deque
from contextlib import ExitStack

import numpy as np
import ml_dtypes

import concourse.bass as bass
import concourse.mybir as mybir
import concourse.tile as tile
from concourse import bacc
from concourse.bass_utils import run_bass_kernel_spmd
from concourse.masks import make_identity

F32 = mybir.dt.float32
BF16 = mybir.dt.bfloat16
DT = BF16
AF = mybir.ActivationFunctionType

N_CORES = 8
B, S, D = 2, 2048, 2048
HQ_TOT, HKV_TOT, HD = 32, 8, 64
ROPE_BASE = 10000.0
TP = N_CORES // B          # 4 cores per batch element
HQ = HQ_TOT // TP          # 8 q heads per core
HKV = HKV_TOT // TP        # 2 kv heads per core
DK = D // 128              # contraction k-tiles (16)
T = S // 512               # t-blocks of 512 q rows (4)
KT = S // 128              # kpos tiles of 128 (16)
CH = 512                   # position-chunk width
QM = 4                     # q-proj m-tiles (head pairs)
SCALE = float(HD) ** -0.5
SHUF_ROPE = [i ^ 16 for i in range(32)]


def build(nc):
    hst = nc.dram_tensor("hst", [128, T * DK * CH], DT, kind="ExternalInput")
    wqt = nc.dram_tensor("wqt", [128, DK * 512], DT, kind="ExternalInput")
    wkvt = nc.dram_tensor("wkvt", [128, DK * 256], DT, kind="ExternalInput")
    wot = nc.dram_tensor("wot", [128, 4 * D], DT, kind="ExternalInput")
    cosd = nc.dram_tensor("cosd", [128, S], DT, kind="ExternalInput")
    sind = nc.dram_tensor("sind", [128, S], DT, kind="ExternalInput")
    lmd = nc.dram_tensor("lmd", [128, KT], F32, kind="ExternalInput")
    outt = nc.dram_tensor("outt", [128, 16 * T * 512], DT, kind="ExternalOutput")

    ctx = ExitStack()
    with tile.TileContext(nc) as tc:
        consts = ctx.enter_context(tc.tile_pool(name="consts", bufs=1))
        big = ctx.enter_context(tc.tile_pool(name="big", bufs=1))
        hsp = ctx.enter_context(tc.tile_pool(name="hsp", bufs=3))
        wp = ctx.enter_context(tc.tile_pool(name="wp", bufs=2))
        qrp = ctx.enter_context(tc.tile_pool(name="qrp", bufs=8))
        pp = ctx.enter_context(tc.tile_pool(name="pp", bufs=2, space="PSUM"))

        # ---------------- lead-in DMAs ----------------
        # wkvt host layout: [K cols (DK*128) | V cols (DK*128)];
        # wqt host layout: m-tile major [(m, k, 128)].
        # Load in first-use order (K, hs0, q0-cols, V, q1-3-cols) so the
        # first projection matmuls start as soon as their k-tiles land.
        wkv_sb = consts.tile([128, DK * 256], DT, name="wkvsb")
        wq_sb = consts.tile([128, DK * 512], DT, name="wqsb")

        hs_tiles = {}

        def load_hs(c, quarters=range(4)):
            # a DMA queue serializes its transfers at ~100GB/s: split the
            # chunk across the sync and gpsimd queues.
            if c in hs_tiles:
                hg = hs_tiles[c]
            else:
                hg = hsp.tile([128, DK * CH], DT, tag="hs", name=f"hs{c}")
                hs_tiles[c] = hg
            for q in quarters:
                eng = nc.sync if q % 2 == 0 else nc.gpsimd
                eng.dma_start(
                    hg[:, q * 2048:(q + 1) * 2048],
                    hst[:, c * DK * CH + q * 2048: c * DK * CH + (q + 1) * 2048])

        # lead-in spread over the three DMA-capable queues in first-use order
        for q in range(4):
            nc.scalar.dma_start(wkv_sb[:, q * 512:(q + 1) * 512],
                                wkvt[:, q * 512:(q + 1) * 512])
            load_hs(0, [q])
        nc.scalar.dma_start(wq_sb[:, 0:2048], wqt[:, 0:2048])
        cos_sb = consts.tile([128, S], DT, name="cossb")
        nc.sync.dma_start(cos_sb[:], cosd[:, :])
        sin_sb = consts.tile([128, S], DT, name="sinsb")
        nc.gpsimd.dma_start(sin_sb[:], sind[:, :])
        nc.scalar.dma_start(wkv_sb[:, 2048:4096], wkvt[:, 2048:4096])
        for m in range(1, QM):
            nc.scalar.dma_start(wq_sb[:, m * 2048:(m + 1) * 2048],
                                wqt[:, m * 2048:(m + 1) * 2048])
        lm_sb = consts.tile([128, KT], F32, name="lmsb")
        nc.sync.dma_start(lm_sb[:], lmd[:, :])
        wo_sb = consts.tile([128, 4 * D], DT, name="wosb")

        # ---------------- resident tensors ----------------
        ZkT = big.tile([128, S], DT, name="ZkT")
        ZkB = big.tile([128, S], DT, name="ZkB")
        nc.gpsimd.memset(ZkT[:], 0.0)
        nc.gpsimd.memset(ZkB[:], 0.0)
        vext = [big.tile([128, 2 * 65], DT, name=f"vext{u}") for u in range(KT)]
        for u in range(KT):
            nc.vector.memset(vext[u][:, 64:65], 1.0)
            nc.vector.memset(vext[u][:, 129:130], 1.0)
        attnT = [big.tile([128, S], DT, name=f"attnT{k}") for k in range(QM)]
        ident = consts.tile([128, 128], DT, name="ident")
        make_identity(nc, ident[:])
        ones64 = consts.tile([1, 64], F32, name="ones64")
        nc.vector.memset(ones64[:], 1.0)
        tri01 = consts.tile([128, 128], DT, name="tri01")
        nc.vector.memset(tri01[:], 1.0)
        nc.gpsimd.affine_select(
            out=tri01[:], in_=tri01[:], compare_op=mybir.AluOpType.is_ge,
            fill=0.0, base=0, channel_multiplier=-1, pattern=[[1, 128]])
        # prime the ACT exp table while the PE is still in lead-in
        warm = wp.tile([1, 32], F32, tag="warm")
        nc.vector.memset(warm[:], 0.0)
        nc.scalar.activation(warm[:], warm[:], AF.Exp)

        qr_tiles = {}

        # ---------------- rope ----------------
        def rope_into(dst, src, col0):
            P = wp.tile([128, CH], DT, tag="ropeP")
            nc.vector.stream_shuffle(P[:], src[:], mask=SHUF_ROPE)
            m1 = wp.tile([128, CH], DT, tag="ropem1")
            nc.vector.tensor_mul(m1[:], src[:], cos_sb[:, col0:col0 + CH])
            m2 = wp.tile([128, CH], DT, tag="ropem2")
            nc.vector.tensor_mul(m2[:], P[:], sin_sb[:, col0:col0 + CH])
            nc.vector.tensor_add(dst[:], m1[:], m2[:])

        # ---------------- per-chunk projection units ----------------
        def proj_mms(pj, wsl_fn, hs):
            for k in range(DK):
                nc.tensor.matmul(pj[:], wsl_fn(k), hs[:, k * CH:(k + 1) * CH],
                                 start=(k == 0), stop=(k == DK - 1))

        def unit_k(c):
            def emit():
                col0 = c * CH
                pj = pp.tile([128, CH], F32, tag="pj", name=f"pk{c}")
                proj_mms(pj, lambda k: wkv_sb[:, k * 128:(k + 1) * 128],
                         hs_tiles[c])
                kc = wp.tile([128, CH], DT, tag="kc", name=f"kc{c}")
                nc.vector.tensor_copy(kc[:], pj[:])
                kr = wp.tile([128, CH], DT, tag="kr", name=f"kr{c}")
                rope_into(kr, kc, col0)
                nc.vector.tensor_copy(ZkT[0:64, col0:col0 + CH], kr[0:64, :])
                nc.vector.tensor_copy(ZkB[64:128, col0:col0 + CH], kr[64:128, :])
            return emit

        def unit_v(c):
            def emit():
                pj = pp.tile([128, CH], F32, tag="pj", name=f"pv{c}")
                proj_mms(pj, lambda k: wkv_sb[:, 2048 + k * 128:
                                              2048 + (k + 1) * 128],
                         hs_tiles[c])
                vc = wp.tile([128, CH], DT, tag="vc", name=f"vc{c}")
                nc.vector.tensor_copy(vc[:], pj[:])
                for uu in range(4):
                    u = 4 * c + uu
                    ps_t = pp.tile([128, 128], DT, tag="pj", name=f"pt{u}")
                    nc.tensor.transpose(ps_t[:], vc[:, uu * 128:(uu + 1) * 128],
                                        ident[:])
                    dst = vext[u][:].rearrange("p (j cc) -> p j cc", j=2)[:, :, 0:64]
                    nc.vector.tensor_copy(
                        dst, ps_t[:].rearrange("p (j cc) -> p j cc", j=2))
            return emit

        def unit_q(c, m):
            def emit():
                col0 = c * CH
                pj = pp.tile([128, CH], F32, tag="pj", name=f"pq{c}_{m}")
                proj_mms(pj, lambda k: wq_sb[:, m * 2048 + k * 128:
                                             m * 2048 + (k + 1) * 128],
                         hs_tiles[c])
                qc = wp.tile([128, CH], DT, tag="qc", name=f"qc{c}_{m}")
                nc.vector.tensor_copy(qc[:], pj[:])
                qr = qrp.tile([128, CH], DT, tag="qr", name=f"qr{c}_{m}")
                rope_into(qr, qc, col0)
                qr_tiles[(c, m)] = qr
            return emit

        def chunk_units(c):
            return ([unit_k(c), unit_v(c)] + [unit_q(c, m) for m in range(QM)])

        # ---------------- o-projection units ----------------
        def _osb_out(t, mD, po):
            osb = wp.tile([128, 512], DT, tag="osb", bufs=4)
            if t == T - 1:
                # tail: both engines idle; split-copy halves the latency
                nc.scalar.copy(osb[:, 0:256], po[:, 0:256])
                nc.vector.tensor_copy(osb[:, 256:512], po[:, 256:512])
            elif mD % 2 and t != 2:
                # C(2) runs inside the ACT-tight window 3: keep its copies
                # off the scalar queue there.
                nc.scalar.copy(osb[:], po[:])
            else:
                nc.vector.tensor_copy(osb[:], po[:])
            oeng = nc.sync if mD % 2 == 0 else nc.gpsimd
            oeng.dma_start(
                outt[:, (mD * T + t) * 512:(mD * T + t) * 512 + 512],
                osb[:])

        def unit_oproj(t, mD):
            def emit():
                po = pp.tile([128, 512], F32, tag="pj", name=f"po{t}_{mD}")
                for k in range(QM):
                    nc.tensor.matmul(
                        po[:],
                        wo_sb[:, k * D + mD * 128:k * D + (mD + 1) * 128],
                        attnT[k][:, t * 512:(t + 1) * 512],
                        start=(k == 0), stop=(k == QM - 1))
                _osb_out(t, mD, po)
            return emit

        # split oproj for the tail: accumulate k=0..2 (pairs already normed)
        # while the last pair's normalize chain runs, finish k=3 after.
        po_pre = {}

        def oproj_pre(t, mD):
            def emit():
                po = pp.tile([128, 512], F32, tag="pj", name=f"poP{t}_{mD}")
                for k in range(QM - 1):
                    nc.tensor.matmul(
                        po[:],
                        wo_sb[:, k * D + mD * 128:k * D + (mD + 1) * 128],
                        attnT[k][:, t * 512:(t + 1) * 512],
                        start=(k == 0), stop=False)
                po_pre[(t, mD)] = po
            return emit

        def oproj_fin(t, mD):
            def emit():
                po = po_pre.pop((t, mD))
                k = QM - 1
                nc.tensor.matmul(
                    po[:],
                    wo_sb[:, k * D + mD * 128:k * D + (mD + 1) * 128],
                    attnT[k][:, t * 512:(t + 1) * 512],
                    start=False, stop=True)
                _osb_out(t, mD, po)
            return emit

        # ---------------- attention steps for one t-block ----------------
        def a_steps(t):
            pairs = []
            nu = 4 * t + 4
            for mp in range(QM):
                steps = []
                pairs.append(steps)
                st = {}
                pa = [None, None]

                def mk_pa(mp=mp, pa=pa):
                    def f():
                        pa[0] = pp.tile([65, 512], F32, tag="pa",
                                        name=f"pa0_{t}_{mp}")
                        pa[1] = pp.tile([65, 512], F32, tag="pa",
                                        name=f"pa1_{t}_{mp}")
                    return f

                def mk_S(u, mp=mp, st=st):
                    def f():
                        w = u - 4 * t
                        c0 = 0 if w < 0 else 128 * w
                        qr = qr_tiles[(t, mp)]
                        s2 = pp.tile([128, 1024], F32, tag="s2",
                                     name=f"s2_{t}_{mp}_{u}")
                        nc.tensor.matmul(s2[:, c0:512],
                                         ZkT[:, u * 128:(u + 1) * 128],
                                         qr[:, c0:512], start=True, stop=True)
                        nc.tensor.matmul(s2[:, 512 + c0:1024],
                                         ZkB[:, u * 128:(u + 1) * 128],
                                         qr[:, c0:512], start=True, stop=True)
                        st[u] = s2
                    return f

                def mk_EP(u, mp=mp, st=st, pa=pa):
                    def f():
                        w = u - 4 * t
                        c0 = 0 if w < 0 else 128 * w
                        s2 = st.pop(u)
                        pr2 = wp.tile([128, 1024], DT, tag="pr", bufs=4,
                                      name=f"pr{t}_{mp}_{u}")
                        if c0 == 0:
                            nc.scalar.activation(pr2[:], s2[:], AF.Exp,
                                                 bias=lm_sb[:, u:u + 1],
                                                 scale=SCALE)
                        else:
                            view = lambda x: x[:].rearrange(
                                "p (h q) -> p h q", h=2)[:, :, c0:512]
                            nc.scalar.activation(view(pr2), view(s2), AF.Exp,
                                                 bias=lm_sb[:, u:u + 1],
                                                 scale=SCALE)
                        if w >= 0:
                            for hh in range(2):
                                base = hh * 512 + c0
                                nc.vector.tensor_mul(pr2[:, base:base + 128],
                                                     pr2[:, base:base + 128],
                                                     tri01[:])
                        for hh in range(2):
                            nc.tensor.matmul(
                                pa[hh][:, c0:512],
                                vext[u][:, hh * 65:(hh + 1) * 65],
                                pr2[:, hh * 512 + c0:(hh + 1) * 512],
                                start=(u == 0), stop=(u == nu - 1))
                    return f

                def mk_norm(mp=mp, pa=pa):
                    def f():
                        # normalize: denominator row hops to partition 0 via
                        # a DVE shifted copy (gpsimd/custom-DVE ops mishandle
                        # partition-offset views on HW, plain DVE copies
                        # don't), then recip -> broadcast -> scale.
                        tail = (t == T - 1 and mp == QM - 1)
                        for hh in range(2):
                            a_sb = wp.tile([65, 512], F32, tag="asb", bufs=4,
                                           name=f"a{hh}_{t}_{mp}")
                            if tail:
                                nc.scalar.copy(a_sb[:], pa[hh][:])
                            else:
                                nc.vector.tensor_copy(a_sb[:], pa[hh][:])
                            rcp = wp.tile([1, 512], F32, tag="rcp", bufs=4,
                                          name=f"rcp{hh}_{t}_{mp}")
                            nc.vector.tensor_copy(rcp[0:1, :],
                                                  a_sb[64:65, :])
                            nc.vector.reciprocal_approx_fast(
                                rcp[0:1, :], rcp[0:1, :])
                            if tail:
                                # PE outer-product broadcast keeps the array
                                # busy (HAM warm) through the tail chain
                                bb = pp.tile([64, 512], F32, tag="s2",
                                             name=f"bbp{hh}_{t}_{mp}")
                                nc.tensor.matmul(bb[:], ones64[0:1, :],
                                                 rcp[0:1, :],
                                                 start=True, stop=True)
                            else:
                                bb = wp.tile([64, 512], F32, tag="bb")
                                nc.gpsimd.partition_broadcast(bb[:],
                                                              rcp[0:1, :])
                            nc.vector.tensor_mul(
                                attnT[mp][hh * 64:hh * 64 + 64,
                                          t * 512:(t + 1) * 512],
                                a_sb[0:64, :], bb[:])
                    return f

                def fuse(*fs):
                    def f():
                        for g in fs:
                            g()
                    return f

                steps.append(fuse(mk_pa(), mk_S(0)))
                steps.append(mk_S(1))
                for u in range(2, nu):
                    steps.append(fuse(mk_EP(u - 2), mk_S(u)))
                steps.append(mk_EP(nu - 2))
                if t == T - 1 and mp == QM - 1:
                    steps.append(fuse(mk_EP(nu - 1), oproj_pre(t, 0),
                                      oproj_pre(t, 1), mk_norm()))
                else:
                    steps.append(fuse(mk_EP(nu - 1), mk_norm()))
            return pairs

        # ---------------- windowed schedule ----------------
        def run_window(t, pre=()):
            fillers = deque()
            # build filler list: chunk t+1 projections and oproj of t-1,
            # interleaved so pj-tag allocations alternate users
            proj = list(pre) + (chunk_units(t + 1) if t + 1 < T else [])
            opro = [unit_oproj(t - 1, mD) for mD in range(16)] if t >= 1 else []
            # merge proportionally
            na, nb = len(proj), len(opro)
            ia = ib = 0
            while ia < na or ib < nb:
                if ib * max(na, 1) <= ia * max(nb, 1) and ib < nb:
                    fillers.append(opro[ib]); ib += 1
                elif ia < na:
                    fillers.append(proj[ia]); ia += 1
                else:
                    fillers.append(opro[ib]); ib += 1
            pairs = a_steps(t)
            if t + 2 < T:
                load_hs(t + 2)
            if t == T - 1:
                # drain all fillers before the last pair so its fused oproj
                # prologue owns the pj PSUM slots (and PE stays warm through
                # the final normalize chain)
                head = [s for pr in pairs[:-1] for s in pr]
                tail_steps = pairs[-1]
            else:
                head = [s for pr in pairs for s in pr]
                tail_steps = []
            ratio = len(fillers) / max(len(head), 1)
            acc = 0.0
            for stp in head:
                stp()
                acc += ratio
                while acc >= 1.0 and fillers:
                    fillers.popleft()()
                    acc -= 1.0
            while fillers:
                fillers.popleft()()
            for stp in tail_steps:
                stp()

        # lead-in: only the chunk-0 units A(0)'s first head-pair needs;
        # the remaining q-projections and the hs1/wo loads become window-0
        # fillers so attention matmuls start while lead-in DMAs drain.
        unit_k(0)()
        unit_q(0, 0)()
        unit_v(0)()
        pre0 = ([lambda: load_hs(1)]
                + [unit_q(0, m) for m in range(1, QM)]
                + [lambda: nc.gpsimd.dma_start(wo_sb[:], wot[:, :])])
        run_window(0, pre=pre0)
        for t in range(1, T):
            run_window(t)
        oproj_fin(T - 1, 0)()
        oproj_fin(T - 1, 1)()
        for mD in range(2, 16):
            unit_oproj(T - 1, mD)()
        import os
        if os.environ.get("DBG_DUMP"):
            dzt = nc.dram_tensor("dbg_zkt", [128, S], DT, kind="ExternalOutput")
            dzb = nc.dram_tensor("dbg_zkb", [128, S], DT, kind="ExternalOutput")
            nc.sync.dma_start(dzt[:, :], ZkT[:])
            nc.sync.dma_start(dzb[:, :], ZkB[:])
            dve = nc.dram_tensor("dbg_vext", [128, KT * 130], DT,
                                 kind="ExternalOutput")
            for u in range(KT):
                nc.sync.dma_start(dve[:, u * 130:(u + 1) * 130], vext[u][:])
            dat = nc.dram_tensor("dbg_attnT", [128, QM * S], DT,
                                 kind="ExternalOutput")
            for k in range(QM):
                nc.sync.dma_start(dat[:, k * S:(k + 1) * S], attnT[k][:])
        ctx.close()
    return nc


def _host_prep(hidden_states, attention_mask, Wq, Wk, Wv, Wo):
    bf16 = ml_dtypes.bfloat16
    hs = np.asarray(hidden_states, np.float32)
    am = np.asarray(attention_mask)
    Wq = np.asarray(Wq, np.float32)
    Wk = np.asarray(Wk, np.float32)
    Wv = np.asarray(Wv, np.float32)
    Wo = np.asarray(Wo, np.float32)

    # rope tables in the 16-interleaved head-dim layout
    inv = 1.0 / (ROPE_BASE ** (np.arange(0, HD, 2, dtype=np.float64) / HD))
    freqs = np.arange(S, dtype=np.float64)[:, None] * inv[None, :]  # [S, 32]
    cosf = np.cos(freqs)
    sinf = np.sin(freqs)
    fidx = np.array([r % 16 + 16 * (r // 32) for r in range(64)])
    sign = np.array([-1.0 if (r // 16) % 2 == 0 else 1.0 for r in range(64)])
    cos64 = cosf[:, fidx].T                      # [64, S]
    sin64 = (sinf[:, fidx] * sign[None, :]).T    # [64, S]
    cos128 = np.concatenate([cos64, cos64], 0).astype(np.float32)
    sin128 = np.concatenate([sin64, sin64], 0).astype(np.float32)

    def perm16(w):  # [D, 64] head-dim cols -> 16-interleaved e/o layout
        return np.concatenate(
            [w[:, 0:32:2], w[:, 1:32:2], w[:, 32:64:2], w[:, 33:64:2]], axis=1)

    def tile_rows(w, ncols):  # [DK*128, ncols] -> [128, DK*ncols]
        return np.ascontiguousarray(
            w.reshape(DK, 128, ncols).transpose(1, 0, 2).reshape(128, DK * ncols))

    hst_b = []
    for b in range(B):
        hb = hs[b].reshape(T, CH, DK, 128).transpose(3, 0, 2, 1)
        hst_b.append(np.ascontiguousarray(hb.reshape(128, T * DK * CH)).astype(bf16))
    lm_b = []
    for b in range(B):
        lm = np.where(am[b] > 0, 0.0, -1e30).astype(np.float32)
        lm_b.append(np.ascontiguousarray(lm.reshape(KT, 128).T))

    in_maps = []
    for core in range(N_CORES):
        b, g = core // TP, core % TP
        h0 = g * HQ
        kv0 = g * HKV
        # q: m-tile mp = [head h0+mp | head h0+mp+4], both perm16'd
        wq_c = np.concatenate(
            [np.concatenate([perm16(Wq[:, (h0 + mp) * HD:(h0 + mp + 1) * HD]),
                             perm16(Wq[:, (h0 + mp + 4) * HD:(h0 + mp + 5) * HD])],
                            axis=1)
             for mp in range(QM)], axis=1)                      # [D, 512]
        wk_c = np.concatenate(
            [perm16(Wk[:, (kv0 + j) * HD:(kv0 + j + 1) * HD]) for j in range(2)],
            axis=1)                                             # [D, 128]
        wv_c = np.concatenate(
            [Wv[:, (kv0 + j) * HD:(kv0 + j + 1) * HD] for j in range(2)], axis=1)
        wkv_c = np.concatenate([wk_c, wv_c], axis=1)            # [D, 256]
        # o: row-block k = [head h0+k | head h0+k+4]
        wo_c = np.concatenate(
            [np.concatenate([Wo[(h0 + k) * HD:(h0 + k + 1) * HD, :],
                             Wo[(h0 + k + 4) * HD:(h0 + k + 5) * HD, :]], axis=0)
             for k in range(QM)], axis=0)                       # [512, D]
        wqt = np.concatenate(
            [tile_rows(np.ascontiguousarray(wq_c[:, m * 128:(m + 1) * 128]), 128)
             for m in range(QM)], axis=1)
        wkvt = np.concatenate(
            [tile_rows(wk_c, 128), tile_rows(np.ascontiguousarray(wv_c), 128)],
            axis=1)
        in_maps.append({
            "hst": hst_b[b],
            "wqt": np.ascontiguousarray(wqt).astype(bf16),
            "wkvt": np.ascontiguousarray(wkvt).astype(bf16),
            "wot": np.ascontiguousarray(
                wo_c.reshape(QM, 128, D).transpose(1, 0, 2)
                .reshape(128, QM * D)).astype(bf16),
            "cosd": cos128.astype(bf16),
            "sind": sin128.astype(bf16),
            "lmd": lm_b[b],
        })
    return in_maps


_NC_CACHE = {}


def _get_nc():
    if "nc" not in _NC_CACHE:
        nc = bacc.Bacc("TRN2", target_bir_lowering=False, num_devices=N_CORES)
        build(nc)
        nc.compile()
        _NC_CACHE["nc"] = nc
    return _NC_CACHE["nc"]


def kernel(hidden_states, attention_mask, Wq, Wk, Wv, Wo):
    nc = _get_nc()
    in_maps = _host_prep(hidden_states, attention_mask, Wq, Wk, Wv, Wo)
    res = run_bass_kernel_spmd(nc, in_maps, list(range(N_CORES)))
    out = np.zeros((B, S, D), np.float32)
    for core, r in enumerate(res.results):
        o = r["outt"].reshape(128, 16, T, 512).transpose(1, 0, 2, 3)
        out[core // TP] += o.reshape(D, S).T
    return out


# revision 49
# speedup vs baseline: 1.1756x; 1.0118x over previous
"""TRN2 Bass kernel for nn_MultiHeadAttention (GQA + RoPE + causal, dense transformer).

Sharding: tensor-parallel over kv-head groups (TP=4; each core owns 2 kv heads
plus their 8 grouped q heads) x data-parallel over batch (DP=2) -> 8 cores.
The host sums the 4 partial o-projection outputs per batch element (the TP
all-reduce) and transposes back.

Per-core schedule (windowed software pipeline over 4 t-blocks of 512 q rows):
  W(-1): weight/const DMAs + chunk-0 projections.
  W(t):  attention for t-block t (scores -> exp -> PV, with per-pair softmax
         normalize folded in) interleaved at u-step granularity with chunk
         t+1 projections and the o-projection of t-block t-1, so the PE queue
         always has independent matmuls to chew while the scalar engine exps.

Key layout tricks vs the straightforward version:
  - Head pairing (m, m+4): q-proj m-tile stacks a kv0-group head (rows 0-63)
    with a kv1-group head (rows 64-127); score matmuls use half-zero weights
    ZkT (K_kv0 top) / ZkB (K_kv1 bottom) so no q duplication is needed.
  - RoPE head-dim layout [e0-15|o0-15|e16-31|o16-31]: the rotate-half partner
    is p XOR 16, an intra-quadrant move done by one DVE stream_shuffle
    (no SBUF-SBUF DMAs on the scalar queue).
  - V transposes via the DMA XBAR (dma_start_transpose) straight into vext,
    with host-interleaved V columns to match the 3D-output mapping.
  - Softmax denominators ride the PV matmul as a 65th ones-column; normalize
    uses reciprocal_approx_fast in place + partition_broadcast from the
    denominator row (no DMA round-trips).
  - All HBM loads are host-pre-tiled to [128, *] contiguous layouts.

Numerics: bf16 matmul paths with fp32 PSUM accumulation; softmax denominators
and normalization in f32.
"""
from collections import deque
from contextlib import ExitStack

import numpy as np
import ml_dtypes

import concourse.bass as bass
import concourse.mybir as mybir
import concourse.tile as tile
from concourse import bacc
from concourse.bass_utils import run_bass_kernel_spmd
from concourse.masks import make_identity

F32 = mybir.dt.float32
BF16 = mybir.dt.bfloat16
DT = BF16
AF = mybir.ActivationFunctionType

N_CORES = 8
B, S, D = 2, 2048, 2048
HQ_TOT, HKV_TOT, HD = 32, 8, 64
ROPE_BASE = 10000.0
TP = N_CORES // B          # 4 cores per batch element
HQ = HQ_TOT // TP          # 8 q heads per core
HKV = HKV_TOT // TP        # 2 kv heads per core
DK = D // 128              # contraction k-tiles (16)
T = S // 512               # t-blocks of 512 q rows (4)
KT = S // 128              # kpos tiles of 128 (16)
CH = 512                   # position-chunk width
QM = 4                     # q-proj m-tiles (head pairs)
SCALE = float(HD) ** -0.5
SHUF_ROPE = [i ^ 16 for i in range(32)]


def build(nc):
    hst = nc.dram_tensor("hst", [128, T * DK * CH], DT, kind="ExternalInput")
    wqt = nc.dram_tensor("wqt", [128, DK * 512], DT, kind="ExternalInput")
    wkvt = nc.dram_tensor("wkvt", [128, DK * 256], DT, kind="ExternalInput")
    wot = nc.dram_tensor("wot", [128, 4 * D], DT, kind="ExternalInput")
    cosd = nc.dram_tensor("cosd", [128, S], DT, kind="ExternalInput")
    sind = nc.dram_tensor("sind", [128, S], DT, kind="ExternalInput")
    lmd = nc.dram_tensor("lmd", [128, KT], F32, kind="ExternalInput")
    outt = nc.dram_tensor("outt", [128, 16 * T * 512], DT, kind="ExternalOutput")

    ctx = ExitStack()
    with tile.TileContext(nc) as tc:
        consts = ctx.enter_context(tc.tile_pool(name="consts", bufs=1))
        big = ctx.enter_context(tc.tile_pool(name="big", bufs=1))
        hsp = ctx.enter_context(tc.tile_pool(name="hsp", bufs=3))
        wp = ctx.enter_context(tc.tile_pool(name="wp", bufs=2))
        qrp = ctx.enter_context(tc.tile_pool(name="qrp", bufs=8))
        pp = ctx.enter_context(tc.tile_pool(name="pp", bufs=2, space="PSUM"))

        # ---------------- lead-in DMAs ----------------
        # wkvt host layout: [K cols (DK*128) | V cols (DK*128)];
        # wqt host layout: m-tile major [(m, k, 128)].
        # Load in first-use order (K, hs0, q0-cols, V, q1-3-cols) so the
        # first projection matmuls start as soon as their k-tiles land.
        wkv_sb = consts.tile([128, DK * 256], DT, name="wkvsb")
        wq_sb = consts.tile([128, DK * 512], DT, name="wqsb")

        hs_tiles = {}

        def load_hs(c, quarters=range(4)):
            # a DMA queue serializes its transfers at ~100GB/s: split the
            # chunk across the sync and gpsimd queues.
            if c in hs_tiles:
                hg = hs_tiles[c]
            else:
                hg = hsp.tile([128, DK * CH], DT, tag="hs", name=f"hs{c}")
                hs_tiles[c] = hg
            for q in quarters:
                eng = nc.sync if q % 2 == 0 else nc.gpsimd
                eng.dma_start(
                    hg[:, q * 2048:(q + 1) * 2048],
                    hst[:, c * DK * CH + q * 2048: c * DK * CH + (q + 1) * 2048])

        # lead-in spread over the three DMA-capable queues in first-use order
        for q in range(4):
            nc.scalar.dma_start(wkv_sb[:, q * 512:(q + 1) * 512],
                                wkvt[:, q * 512:(q + 1) * 512])
            load_hs(0, [q])
        nc.scalar.dma_start(wq_sb[:, 0:2048], wqt[:, 0:2048])
        cos_sb = consts.tile([128, S], DT, name="cossb")
        nc.sync.dma_start(cos_sb[:], cosd[:, :])
        sin_sb = consts.tile([128, S], DT, name="sinsb")
        nc.gpsimd.dma_start(sin_sb[:], sind[:, :])
        nc.scalar.dma_start(wkv_sb[:, 2048:4096], wkvt[:, 2048:4096])
        for m in range(1, QM):
            nc.scalar.dma_start(wq_sb[:, m * 2048:(m + 1) * 2048],
                                wqt[:, m * 2048:(m + 1) * 2048])
        lm_sb = consts.tile([128, KT], F32, name="lmsb")
        nc.sync.dma_start(lm_sb[:], lmd[:, :])
        wo_sb = consts.tile([128, 4 * D], DT, name="wosb")

        # ---------------- resident tensors ----------------
        ZkT = big.tile([128, S], DT, name="ZkT")
        ZkB = big.tile([128, S], DT, name="ZkB")
        nc.gpsimd.memset(ZkT[:], 0.0)
        nc.gpsimd.memset(ZkB[:], 0.0)
        vext = [big.tile([128, 2 * 65], DT, name=f"vext{u}") for u in range(KT)]
        for u in range(KT):
            nc.vector.memset(vext[u][:, 64:65], 1.0)
            nc.vector.memset(vext[u][:, 129:130], 1.0)
        attnT = [big.tile([128, S], DT, name=f"attnT{k}") for k in range(QM)]
        ident = consts.tile([128, 128], DT, name="ident")
        make_identity(nc, ident[:])
        ones64 = consts.tile([1, 64], F32, name="ones64")
        nc.vector.memset(ones64[:], 1.0)
        tri01 = consts.tile([128, 128], DT, name="tri01")
        nc.vector.memset(tri01[:], 1.0)
        nc.gpsimd.affine_select(
            out=tri01[:], in_=tri01[:], compare_op=mybir.AluOpType.is_ge,
            fill=0.0, base=0, channel_multiplier=-1, pattern=[[1, 128]])
        # prime the ACT exp table while the PE is still in lead-in
        warm = wp.tile([1, 32], F32, tag="warm")
        nc.vector.memset(warm[:], 0.0)
        nc.scalar.activation(warm[:], warm[:], AF.Exp)

        qr_tiles = {}

        # ---------------- rope ----------------
        def rope_into(dst, src, col0):
            P = wp.tile([128, CH], DT, tag="ropeP")
            nc.vector.stream_shuffle(P[:], src[:], mask=SHUF_ROPE)
            m1 = wp.tile([128, CH], DT, tag="ropem1")
            nc.vector.tensor_mul(m1[:], src[:], cos_sb[:, col0:col0 + CH])
            m2 = wp.tile([128, CH], DT, tag="ropem2")
            nc.vector.tensor_mul(m2[:], P[:], sin_sb[:, col0:col0 + CH])
            nc.vector.tensor_add(dst[:], m1[:], m2[:])

        # ---------------- per-chunk projection units ----------------
        def proj_mms(pj, wsl_fn, hs):
            for k in range(DK):
                nc.tensor.matmul(pj[:], wsl_fn(k), hs[:, k * CH:(k + 1) * CH],
                                 start=(k == 0), stop=(k == DK - 1))

        def unit_k(c):
            def emit():
                col0 = c * CH
                pj = pp.tile([128, CH], F32, tag="pj", name=f"pk{c}")
                proj_mms(pj, lambda k: wkv_sb[:, k * 128:(k + 1) * 128],
                         hs_tiles[c])
                kc = wp.tile([128, CH], DT, tag="kc", name=f"kc{c}")
                nc.vector.tensor_copy(kc[:], pj[:])
                kr = wp.tile([128, CH], DT, tag="kr", name=f"kr{c}")
                rope_into(kr, kc, col0)
                nc.vector.tensor_copy(ZkT[0:64, col0:col0 + CH], kr[0:64, :])
                nc.vector.tensor_copy(ZkB[64:128, col0:col0 + CH], kr[64:128, :])
            return emit

        def unit_v(c):
            def emit():
                pj = pp.tile([128, CH], F32, tag="pj", name=f"pv{c}")
                proj_mms(pj, lambda k: wkv_sb[:, 2048 + k * 128:
                                              2048 + (k + 1) * 128],
                         hs_tiles[c])
                vc = wp.tile([128, CH], DT, tag="vc", name=f"vc{c}")
                nc.vector.tensor_copy(vc[:], pj[:])
                for uu in range(4):
                    u = 4 * c + uu
                    ps_t = pp.tile([128, 128], DT, tag="pj", name=f"pt{u}")
                    nc.tensor.transpose(ps_t[:], vc[:, uu * 128:(uu + 1) * 128],
                                        ident[:])
                    dst = vext[u][:].rearrange("p (j cc) -> p j cc", j=2)[:, :, 0:64]
                    nc.vector.tensor_copy(
                        dst, ps_t[:].rearrange("p (j cc) -> p j cc", j=2))
            return emit

        def unit_q(c, m):
            def emit():
                col0 = c * CH
                pj = pp.tile([128, CH], F32, tag="pj", name=f"pq{c}_{m}")
                proj_mms(pj, lambda k: wq_sb[:, m * 2048 + k * 128:
                                             m * 2048 + (k + 1) * 128],
                         hs_tiles[c])
                qc = wp.tile([128, CH], DT, tag="qc", name=f"qc{c}_{m}")
                nc.vector.tensor_copy(qc[:], pj[:])
                qr = qrp.tile([128, CH], DT, tag="qr", name=f"qr{c}_{m}")
                rope_into(qr, qc, col0)
                qr_tiles[(c, m)] = qr
            return emit

        def chunk_units(c):
            return ([unit_k(c), unit_v(c)] + [unit_q(c, m) for m in range(QM)])

        # ---------------- o-projection units ----------------
        def _osb_out(t, mD, po):
            osb = wp.tile([128, 512], DT, tag="osb", bufs=4)
            if t == T - 1:
                # tail: both engines idle; split-copy halves the latency
                nc.scalar.copy(osb[:, 0:256], po[:, 0:256])
                nc.vector.tensor_copy(osb[:, 256:512], po[:, 256:512])
            elif mD % 2 and t != 2:
                # C(2) runs inside the ACT-tight window 3: keep its copies
                # off the scalar queue there.
                nc.scalar.copy(osb[:], po[:])
            else:
                nc.vector.tensor_copy(osb[:], po[:])
            oeng = nc.sync if mD % 2 == 0 else nc.gpsimd
            oeng.dma_start(
                outt[:, (mD * T + t) * 512:(mD * T + t) * 512 + 512],
                osb[:])

        def unit_oproj(t, mD):
            def emit():
                po = pp.tile([128, 512], F32, tag="pj", name=f"po{t}_{mD}")
                for k in range(QM):
                    nc.tensor.matmul(
                        po[:],
                        wo_sb[:, k * D + mD * 128:k * D + (mD + 1) * 128],
                        attnT[k][:, t * 512:(t + 1) * 512],
                        start=(k == 0), stop=(k == QM - 1))
                _osb_out(t, mD, po)
            return emit

        # split oproj for the tail: accumulate k=0..2 (pairs already normed)
        # while the last pair's normalize chain runs, finish k=3 after.
        po_pre = {}

        def oproj_pre(t, mD):
            def emit():
                po = pp.tile([128, 512], F32, tag="pj", name=f"poP{t}_{mD}")
                for k in range(QM - 1):
                    nc.tensor.matmul(
                        po[:],
                        wo_sb[:, k * D + mD * 128:k * D + (mD + 1) * 128],
                        attnT[k][:, t * 512:(t + 1) * 512],
                        start=(k == 0), stop=False)
                po_pre[(t, mD)] = po
            return emit

        def oproj_fin(t, mD):
            def emit():
                po = po_pre.pop((t, mD))
                k = QM - 1
                nc.tensor.matmul(
                    po[:],
                    wo_sb[:, k * D + mD * 128:k * D + (mD + 1) * 128],
                    attnT[k][:, t * 512:(t + 1) * 512],
                    start=False, stop=True)
                _osb_out(t, mD, po)
            return emit

        # ---------------- attention steps for one t-block ----------------
        def a_steps(t):
            pairs = []
            nu = 4 * t + 4
            for mp in range(QM):
                steps = []
                pairs.append(steps)
                st = {}
                pa = [None, None]

                def mk_pa(mp=mp, pa=pa):
                    def f():
                        pa[0] = pp.tile([65, 512], F32, tag="pa",
                                        name=f"pa0_{t}_{mp}")
                        pa[1] = pp.tile([65, 512], F32, tag="pa",
                                        name=f"pa1_{t}_{mp}")
                    return f

                def mk_S(u, mp=mp, st=st):
                    def f():
                        w = u - 4 * t
                        c0 = 0 if w < 0 else 128 * w
                        qr = qr_tiles[(t, mp)]
                        s2 = pp.tile([128, 1024], F32, tag="s2",
                                     name=f"s2_{t}_{mp}_{u}")
                        nc.tensor.matmul(s2[:, c0:512],
                                         ZkT[:, u * 128:(u + 1) * 128],
                                         qr[:, c0:512], start=True, stop=True)
                        nc.tensor.matmul(s2[:, 512 + c0:1024],
                                         ZkB[:, u * 128:(u + 1) * 128],
                                         qr[:, c0:512], start=True, stop=True)
                        st[u] = s2
                    return f

                def mk_EP(u, mp=mp, st=st, pa=pa):
                    def f():
                        w = u - 4 * t
                        c0 = 0 if w < 0 else 128 * w
                        s2 = st.pop(u)
                        pr2 = wp.tile([128, 1024], DT, tag="pr", bufs=4,
                                      name=f"pr{t}_{mp}_{u}")
                        if c0 == 0:
                            nc.scalar.activation(pr2[:], s2[:], AF.Exp,
                                                 bias=lm_sb[:, u:u + 1],
                                                 scale=SCALE)
                        else:
                            view = lambda x: x[:].rearrange(
                                "p (h q) -> p h q", h=2)[:, :, c0:512]
                            nc.scalar.activation(view(pr2), view(s2), AF.Exp,
                                                 bias=lm_sb[:, u:u + 1],
                                                 scale=SCALE)
                        if w >= 0:
                            for hh in range(2):
                                base = hh * 512 + c0
                                nc.vector.tensor_mul(pr2[:, base:base + 128],
                                                     pr2[:, base:base + 128],
                                                     tri01[:])
                        for hh in range(2):
                            nc.tensor.matmul(
                                pa[hh][:, c0:512],
                                vext[u][:, hh * 65:(hh + 1) * 65],
                                pr2[:, hh * 512 + c0:(hh + 1) * 512],
                                start=(u == 0), stop=(u == nu - 1))
                    return f

                def mk_norm(mp=mp, pa=pa):
                    def f():
                        # normalize: denominator row hops to partition 0 via
                        # a DVE shifted copy (gpsimd/custom-DVE ops mishandle
                        # partition-offset views on HW, plain DVE copies
                        # don't), then recip -> broadcast -> scale.
                        tail = (t == T - 1 and mp == QM - 1)
                        for hh in range(2):
                            a_sb = wp.tile([65, 512], F32, tag="asb", bufs=4,
                                           name=f"a{hh}_{t}_{mp}")
                            if tail:
                                nc.scalar.copy(a_sb[:], pa[hh][:])
                            else:
                                nc.vector.tensor_copy(a_sb[:], pa[hh][:])
                            rcp = wp.tile([1, 512], F32, tag="rcp", bufs=4,
                                          name=f"rcp{hh}_{t}_{mp}")
                            nc.vector.tensor_copy(rcp[0:1, :],
                                                  a_sb[64:65, :])
                            nc.vector.reciprocal_approx_fast(
                                rcp[0:1, :], rcp[0:1, :])
                            if tail:
                                # PE outer-product broadcast keeps the array
                                # busy (HAM warm) through the tail chain
                                bb = pp.tile([64, 512], F32, tag="s2",
                                             name=f"bbp{hh}_{t}_{mp}")
                                nc.tensor.matmul(bb[:], ones64[0:1, :],
                                                 rcp[0:1, :],
                                                 start=True, stop=True)
                            else:
                                bb = wp.tile([64, 512], F32, tag="bb")
                                nc.gpsimd.partition_broadcast(bb[:],
                                                              rcp[0:1, :])
                            nc.vector.tensor_mul(
                                attnT[mp][hh * 64:hh * 64 + 64,
                                          t * 512:(t + 1) * 512],
                                a_sb[0:64, :], bb[:])
                    return f

                def fuse(*fs):
                    def f():
                        for g in fs:
                            g()
                    return f

                steps.append(fuse(mk_pa(), mk_S(0)))
                steps.append(mk_S(1))
                for u in range(2, nu):
                    steps.append(fuse(mk_EP(u - 2), mk_S(u)))
                steps.append(mk_EP(nu - 2))
                if t == T - 1 and mp == QM - 1:
                    steps.append(fuse(mk_EP(nu - 1), oproj_pre(t, 0),
                                      oproj_pre(t, 1), mk_norm()))
                else:
                    steps.append(fuse(mk_EP(nu - 1), mk_norm()))
            return pairs

        # ---------------- windowed schedule ----------------
        def run_window(t, pre=()):
            fillers = deque()
            # build filler list: chunk t+1 projections and oproj of t-1,
            # interleaved so pj-tag allocations alternate users
            proj = list(pre) + (chunk_units(t + 1) if t + 1 < T else [])
            opro = [unit_oproj(t - 1, mD) for mD in range(16)] if t >= 1 else []
            # merge proportionally
            na, nb = len(proj), len(opro)
            ia = ib = 0
            while ia < na or ib < nb:
                if ib * max(na, 1) <= ia * max(nb, 1) and ib < nb:
                    fillers.append(opro[ib]); ib += 1
                elif ia < na:
                    fillers.append(proj[ia]); ia += 1
                else:
                    fillers.append(opro[ib]); ib += 1
            pairs = a_steps(t)
            if t + 2 < T:
                load_hs(t + 2)
            flat = [s for pr in pairs for s in pr]
            if t == T - 1:
                # drain fillers before the final fused step so its oproj
                # prologue owns the pj PSUM slots (no pj alloc may sit
                # between the prologue and its finish)
                head = flat[:-1]
                tail_steps = flat[-1:]
            else:
                head = flat
                tail_steps = []
            ratio = len(fillers) / max(len(head), 1)
            acc = 0.0
            for stp in head:
                stp()
                acc += ratio
                while acc >= 1.0 and fillers:
                    fillers.popleft()()
                    acc -= 1.0
            while fillers:
                fillers.popleft()()
            for stp in tail_steps:
                stp()

        # lead-in: only the chunk-0 units A(0)'s first head-pair needs;
        # the remaining q-projections and the hs1/wo loads become window-0
        # fillers so attention matmuls start while lead-in DMAs drain.
        unit_k(0)()
        unit_q(0, 0)()
        unit_v(0)()
        pre0 = ([lambda: load_hs(1)]
                + [unit_q(0, m) for m in range(1, QM)]
                + [lambda: nc.scalar.dma_start(wo_sb[:], wot[:, :])])
        run_window(0, pre=pre0)
        for t in range(1, T):
            run_window(t)
        oproj_fin(T - 1, 0)()
        oproj_fin(T - 1, 1)()
        for mD in range(2, 16):
            unit_oproj(T - 1, mD)()
        import os
        if os.environ.get("DBG_DUMP"):
            dzt = nc.dram_tensor("dbg_zkt", [128, S], DT, kind="ExternalOutput")
            dzb = nc.dram_tensor("dbg_zkb", [128, S], DT, kind="ExternalOutput")
            nc.sync.dma_start(dzt[:, :], ZkT[:])
            nc.sync.dma_start(dzb[:, :], ZkB[:])
            dve = nc.dram_tensor("dbg_vext", [128, KT * 130], DT,
                                 kind="ExternalOutput")
            for u in range(KT):
                nc.sync.dma_start(dve[:, u * 130:(u + 1) * 130], vext[u][:])
            dat = nc.dram_tensor("dbg_attnT", [128, QM * S], DT,
                                 kind="ExternalOutput")
            for k in range(QM):
                nc.sync.dma_start(dat[:, k * S:(k + 1) * S], attnT[k][:])
        ctx.close()
    return nc


def _host_prep(hidden_states, attention_mask, Wq, Wk, Wv, Wo):
    bf16 = ml_dtypes.bfloat16
    hs = np.asarray(hidden_states, np.float32)
    am = np.asarray(attention_mask)
    Wq = np.asarray(Wq, np.float32)
    Wk = np.asarray(Wk, np.float32)
    Wv = np.asarray(Wv, np.float32)
    Wo = np.asarray(Wo, np.float32)

    # rope tables in the 16-interleaved head-dim layout
    inv = 1.0 / (ROPE_BASE ** (np.arange(0, HD, 2, dtype=np.float64) / HD))
    freqs = np.arange(S, dtype=np.float64)[:, None] * inv[None, :]  # [S, 32]
    cosf = np.cos(freqs)
    sinf = np.sin(freqs)
    fidx = np.array([r % 16 + 16 * (r // 32) for r in range(64)])
    sign = np.array([-1.0 if (r // 16) % 2 == 0 else 1.0 for r in range(64)])
    cos64 = cosf[:, fidx].T                      # [64, S]
    sin64 = (sinf[:, fidx] * sign[None, :]).T    # [64, S]
    cos128 = np.concatenate([cos64, cos64], 0).astype(np.float32)
    sin128 = np.concatenate([sin64, sin64], 0).astype(np.float32)

    def perm16(w):  # [D, 64] head-dim cols -> 16-interleaved e/o layout
        return np.concatenate(
            [w[:, 0:32:2], w[:, 1:32:2], w[:, 32:64:2], w[:, 33:64:2]], axis=1)

    def tile_rows(w, ncols):  # [DK*128, ncols] -> [128, DK*ncols]
        return np.ascontiguousarray(
            w.reshape(DK, 128, ncols).transpose(1, 0, 2).reshape(128, DK * ncols))

    hst_b = []
    for b in range(B):
        hb = hs[b].reshape(T, CH, DK, 128).transpose(3, 0, 2, 1)
        hst_b.append(np.ascontiguousarray(hb.reshape(128, T * DK * CH)).astype(bf16))
    lm_b = []
    for b in range(B):
        lm = np.where(am[b] > 0, 0.0, -1e30).astype(np.float32)
        lm_b.append(np.ascontiguousarray(lm.reshape(KT, 128).T))

    in_maps = []
    for core in range(N_CORES):
        b, g = core // TP, core % TP
        h0 = g * HQ
        kv0 = g * HKV
        # q: m-tile mp = [head h0+mp | head h0+mp+4], both perm16'd
        wq_c = np.concatenate(
            [np.concatenate([perm16(Wq[:, (h0 + mp) * HD:(h0 + mp + 1) * HD]),
                             perm16(Wq[:, (h0 + mp + 4) * HD:(h0 + mp + 5) * HD])],
                            axis=1)
             for mp in range(QM)], axis=1)                      # [D, 512]
        wk_c = np.concatenate(
            [perm16(Wk[:, (kv0 + j) * HD:(kv0 + j + 1) * HD]) for j in range(2)],
            axis=1)                                             # [D, 128]
        wv_c = np.concatenate(
            [Wv[:, (kv0 + j) * HD:(kv0 + j + 1) * HD] for j in range(2)], axis=1)
        wkv_c = np.concatenate([wk_c, wv_c], axis=1)            # [D, 256]
        # o: row-block k = [head h0+k | head h0+k+4]
        wo_c = np.concatenate(
            [np.concatenate([Wo[(h0 + k) * HD:(h0 + k + 1) * HD, :],
                             Wo[(h0 + k + 4) * HD:(h0 + k + 5) * HD, :]], axis=0)
             for k in range(QM)], axis=0)                       # [512, D]
        wqt = np.concatenate(
            [tile_rows(np.ascontiguousarray(wq_c[:, m * 128:(m + 1) * 128]), 128)
             for m in range(QM)], axis=1)
        wkvt = np.concatenate(
            [tile_rows(wk_c, 128), tile_rows(np.ascontiguousarray(wv_c), 128)],
            axis=1)
        in_maps.append({
            "hst": hst_b[b],
            "wqt": np.ascontiguousarray(wqt).astype(bf16),
            "wkvt": np.ascontiguousarray(wkvt).astype(bf16),
            "wot": np.ascontiguousarray(
                wo_c.reshape(QM, 128, D).transpose(1, 0, 2)
                .reshape(128, QM * D)).astype(bf16),
            "cosd": cos128.astype(bf16),
            "sind": sin128.astype(bf16),
            "lmd": lm_b[b],
        })
    return in_maps


_NC_CACHE = {}


def _get_nc():
    if "nc" not in _NC_CACHE:
        nc = bacc.Bacc("TRN2", target_bir_lowering=False, num_devices=N_CORES)
        build(nc)
        nc.compile()
        _NC_CACHE["nc"] = nc
    return _NC_CACHE["nc"]


def kernel(hidden_states, attention_mask, Wq, Wk, Wv, Wo):
    nc = _get_nc()
    in_maps = _host_prep(hidden_states, attention_mask, Wq, Wk, Wv, Wo)
    res = run_bass_kernel_spmd(nc, in_maps, list(range(N_CORES)))
    out = np.zeros((B, S, D), np.float32)
    for core, r in enumerate(res.results):
        o = r["outt"].reshape(128, 16, T, 512).transpose(1, 0, 2, 3)
        out[core // TP] += o.reshape(D, S).T
    return out


# revision 54
# speedup vs baseline: 1.2069x; 1.0266x over previous
"""TRN2 Bass kernel for nn_MultiHeadAttention (GQA + RoPE + causal, dense transformer).

Sharding: tensor-parallel over kv-head groups (TP=4; each core owns 2 kv heads
plus their 8 grouped q heads) x data-parallel over batch (DP=2) -> 8 cores.
The host sums the 4 partial o-projection outputs per batch element (the TP
all-reduce) and transposes back.

Per-core schedule (windowed software pipeline over 4 t-blocks of 512 q rows):
  W(-1): weight/const DMAs + chunk-0 projections.
  W(t):  attention for t-block t (scores -> exp -> PV, with per-pair softmax
         normalize folded in) interleaved at u-step granularity with chunk
         t+1 projections and the o-projection of t-block t-1, so the PE queue
         always has independent matmuls to chew while the scalar engine exps.

Key layout tricks vs the straightforward version:
  - Head pairing (m, m+4): q-proj m-tile stacks a kv0-group head (rows 0-63)
    with a kv1-group head (rows 64-127); score matmuls use half-zero weights
    ZkT (K_kv0 top) / ZkB (K_kv1 bottom) so no q duplication is needed.
  - RoPE head-dim layout [e0-15|o0-15|e16-31|o16-31]: the rotate-half partner
    is p XOR 16, an intra-quadrant move done by one DVE stream_shuffle
    (no SBUF-SBUF DMAs on the scalar queue).
  - V transposes via the DMA XBAR (dma_start_transpose) straight into vext,
    with host-interleaved V columns to match the 3D-output mapping.
  - Softmax denominators ride the PV matmul as a 65th ones-column; normalize
    uses reciprocal_approx_fast in place + partition_broadcast from the
    denominator row (no DMA round-trips).
  - All HBM loads are host-pre-tiled to [128, *] contiguous layouts.

Numerics: bf16 matmul paths with fp32 PSUM accumulation; softmax denominators
and normalization in f32.
"""
from collections import deque
from contextlib import ExitStack

import numpy as np
import ml_dtypes

import concourse.bass as bass
import concourse.mybir as mybir
import concourse.tile as tile
from concourse import bacc
from concourse.bass_utils import run_bass_kernel_spmd
from concourse.masks import make_identity

F32 = mybir.dt.float32
BF16 = mybir.dt.bfloat16
DT = BF16
AF = mybir.ActivationFunctionType

N_CORES = 8
B, S, D = 2, 2048, 2048
HQ_TOT, HKV_TOT, HD = 32, 8, 64
ROPE_BASE = 10000.0
TP = N_CORES // B          # 4 cores per batch element
HQ = HQ_TOT // TP          # 8 q heads per core
HKV = HKV_TOT // TP        # 2 kv heads per core
DK = D // 128              # contraction k-tiles (16)
T = S // 512               # t-blocks of 512 q rows (4)
KT = S // 128              # kpos tiles of 128 (16)
CH = 512                   # position-chunk width
QM = 4                     # q-proj m-tiles (head pairs)
SCALE = float(HD) ** -0.5
SHUF_ROPE = [i ^ 16 for i in range(32)]


def build(nc):
    hst = nc.dram_tensor("hst", [128, T * DK * CH], DT, kind="ExternalInput")
    wqt = nc.dram_tensor("wqt", [128, DK * 512], DT, kind="ExternalInput")
    wkvt = nc.dram_tensor("wkvt", [128, DK * 256], DT, kind="ExternalInput")
    wot = nc.dram_tensor("wot", [128, 4 * D], DT, kind="ExternalInput")
    cosd = nc.dram_tensor("cosd", [128, S], DT, kind="ExternalInput")
    sind = nc.dram_tensor("sind", [128, S], DT, kind="ExternalInput")
    lmd = nc.dram_tensor("lmd", [128, KT], F32, kind="ExternalInput")
    outt = nc.dram_tensor("outt", [128, 16 * T * 512], DT, kind="ExternalOutput")

    ctx = ExitStack()
    with tile.TileContext(nc) as tc:
        consts = ctx.enter_context(tc.tile_pool(name="consts", bufs=1))
        big = ctx.enter_context(tc.tile_pool(name="big", bufs=1))
        hsp = ctx.enter_context(tc.tile_pool(name="hsp", bufs=3))
        wp = ctx.enter_context(tc.tile_pool(name="wp", bufs=2))
        qrp = ctx.enter_context(tc.tile_pool(name="qrp", bufs=8))
        pp = ctx.enter_context(tc.tile_pool(name="pp", bufs=2, space="PSUM"))

        # ---------------- lead-in DMAs ----------------
        # wkvt host layout: [K cols (DK*128) | V cols (DK*128)];
        # wqt host layout: m-tile major [(m, k, 128)].
        # Load in first-use order (K, hs0, q0-cols, V, q1-3-cols) so the
        # first projection matmuls start as soon as their k-tiles land.
        wkv_sb = consts.tile([128, DK * 256], DT, name="wkvsb")
        wq_sb = consts.tile([128, DK * 512], DT, name="wqsb")

        hs_tiles = {}

        def load_hs(c, quarters=range(4)):
            # a DMA queue serializes its transfers at ~100GB/s: split the
            # chunk across the sync and gpsimd queues.
            if c in hs_tiles:
                hg = hs_tiles[c]
            else:
                hg = hsp.tile([128, DK * CH], DT, tag="hs", name=f"hs{c}")
                hs_tiles[c] = hg
            for q in quarters:
                eng = nc.sync if q % 2 == 0 else nc.gpsimd
                eng.dma_start(
                    hg[:, q * 2048:(q + 1) * 2048],
                    hst[:, c * DK * CH + q * 2048: c * DK * CH + (q + 1) * 2048])

        # lead-in spread over the three DMA-capable queues in first-use order
        for q in range(4):
            nc.scalar.dma_start(wkv_sb[:, q * 512:(q + 1) * 512],
                                wkvt[:, q * 512:(q + 1) * 512])
            load_hs(0, [q])
        nc.scalar.dma_start(wq_sb[:, 0:2048], wqt[:, 0:2048])
        cos_sb = consts.tile([128, S], DT, name="cossb")
        nc.sync.dma_start(cos_sb[:], cosd[:, :])
        sin_sb = consts.tile([128, S], DT, name="sinsb")
        nc.gpsimd.dma_start(sin_sb[:], sind[:, :])
        nc.scalar.dma_start(wkv_sb[:, 2048:4096], wkvt[:, 2048:4096])
        for m in range(1, QM):
            nc.scalar.dma_start(wq_sb[:, m * 2048:(m + 1) * 2048],
                                wqt[:, m * 2048:(m + 1) * 2048])
        lm_sb = consts.tile([128, KT], F32, name="lmsb")
        nc.sync.dma_start(lm_sb[:], lmd[:, :])
        wo_sb = consts.tile([128, 4 * D], DT, name="wosb")

        # ---------------- resident tensors ----------------
        ZkT = big.tile([128, S], DT, name="ZkT")
        ZkB = big.tile([128, S], DT, name="ZkB")
        nc.gpsimd.memset(ZkT[:], 0.0)
        nc.gpsimd.memset(ZkB[:], 0.0)
        vext = [big.tile([128, 2 * 65], DT, name=f"vext{u}") for u in range(KT)]
        for u in range(KT):
            nc.vector.memset(vext[u][:, 64:65], 1.0)
            nc.vector.memset(vext[u][:, 129:130], 1.0)
        attnT = [big.tile([128, S], DT, name=f"attnT{k}") for k in range(QM)]
        ident = consts.tile([128, 128], DT, name="ident")
        make_identity(nc, ident[:])
        ones64 = consts.tile([1, 64], F32, name="ones64")
        nc.vector.memset(ones64[:], 1.0)
        tri01 = consts.tile([128, 128], DT, name="tri01")
        nc.vector.memset(tri01[:], 1.0)
        nc.gpsimd.affine_select(
            out=tri01[:], in_=tri01[:], compare_op=mybir.AluOpType.is_ge,
            fill=0.0, base=0, channel_multiplier=-1, pattern=[[1, 128]])
        # prime the ACT exp table while the PE is still in lead-in
        warm = wp.tile([1, 32], F32, tag="warm")
        nc.vector.memset(warm[:], 0.0)
        nc.scalar.activation(warm[:], warm[:], AF.Exp)

        qr_tiles = {}

        # ---------------- rope ----------------
        def rope_into(dst, src, col0):
            P = wp.tile([128, CH], DT, tag="ropeP")
            nc.vector.stream_shuffle(P[:], src[:], mask=SHUF_ROPE)
            m1 = wp.tile([128, CH], DT, tag="ropem1")
            nc.vector.tensor_mul(m1[:], src[:], cos_sb[:, col0:col0 + CH])
            m2 = wp.tile([128, CH], DT, tag="ropem2")
            nc.vector.tensor_mul(m2[:], P[:], sin_sb[:, col0:col0 + CH])
            nc.vector.tensor_add(dst[:], m1[:], m2[:])

        # ---------------- per-chunk projection units ----------------
        def proj_mms(pj, wsl_fn, hs):
            for k in range(DK):
                nc.tensor.matmul(pj[:], wsl_fn(k), hs[:, k * CH:(k + 1) * CH],
                                 start=(k == 0), stop=(k == DK - 1))

        def unit_k(c):
            def emit():
                col0 = c * CH
                pj = pp.tile([128, CH], F32, tag="pj", name=f"pk{c}")
                proj_mms(pj, lambda k: wkv_sb[:, k * 128:(k + 1) * 128],
                         hs_tiles[c])
                kc = wp.tile([128, CH], DT, tag="kc", name=f"kc{c}")
                nc.vector.tensor_copy(kc[:], pj[:])
                kr = wp.tile([128, CH], DT, tag="kr", name=f"kr{c}")
                rope_into(kr, kc, col0)
                nc.vector.tensor_copy(ZkT[0:64, col0:col0 + CH], kr[0:64, :])
                nc.vector.tensor_copy(ZkB[64:128, col0:col0 + CH], kr[64:128, :])
            return emit

        def unit_v(c):
            def emit():
                pj = pp.tile([128, CH], F32, tag="pj", name=f"pv{c}")
                proj_mms(pj, lambda k: wkv_sb[:, 2048 + k * 128:
                                              2048 + (k + 1) * 128],
                         hs_tiles[c])
                vc = wp.tile([128, CH], DT, tag="vc", name=f"vc{c}")
                nc.vector.tensor_copy(vc[:], pj[:])
                for uu in range(4):
                    u = 4 * c + uu
                    ps_t = pp.tile([128, 128], DT, tag="pj", name=f"pt{u}")
                    nc.tensor.transpose(ps_t[:], vc[:, uu * 128:(uu + 1) * 128],
                                        ident[:])
                    dst = vext[u][:].rearrange("p (j cc) -> p j cc", j=2)[:, :, 0:64]
                    nc.vector.tensor_copy(
                        dst, ps_t[:].rearrange("p (j cc) -> p j cc", j=2))
            return emit

        def unit_q(c, m):
            def emit():
                col0 = c * CH
                pj = pp.tile([128, CH], F32, tag="pj", name=f"pq{c}_{m}")
                proj_mms(pj, lambda k: wq_sb[:, m * 2048 + k * 128:
                                             m * 2048 + (k + 1) * 128],
                         hs_tiles[c])
                qc = wp.tile([128, CH], DT, tag="qc", name=f"qc{c}_{m}")
                nc.vector.tensor_copy(qc[:], pj[:])
                qr = qrp.tile([128, CH], DT, tag="qr", name=f"qr{c}_{m}")
                rope_into(qr, qc, col0)
                qr_tiles[(c, m)] = qr
            return emit

        def chunk_units(c):
            return ([unit_k(c), unit_v(c)] + [unit_q(c, m) for m in range(QM)])

        # ---------------- o-projection units ----------------
        def _osb_out(t, mD, po):
            osb = wp.tile([128, 512], DT, tag="osb", bufs=4)
            if t == T - 1:
                # tail: both engines idle; split-copy halves the latency
                nc.scalar.copy(osb[:, 0:256], po[:, 0:256])
                nc.vector.tensor_copy(osb[:, 256:512], po[:, 256:512])
            elif mD % 2 and t != 2:
                # C(2) runs inside the ACT-tight window 3: keep its copies
                # off the scalar queue there.
                nc.scalar.copy(osb[:], po[:])
            else:
                nc.vector.tensor_copy(osb[:], po[:])
            oeng = nc.sync if mD % 2 == 0 else nc.gpsimd
            oeng.dma_start(
                outt[:, (mD * T + t) * 512:(mD * T + t) * 512 + 512],
                osb[:])

        def unit_oproj(t, mD):
            def emit():
                # at the tail the s2 score slots are free: rotate over both
                # tags for a 4-deep accumulator pipeline
                tag = "s2" if (t == T - 1 and mD % 2) else "pj"
                po = pp.tile([128, 512], F32, tag=tag, name=f"po{t}_{mD}")
                for k in range(QM):
                    nc.tensor.matmul(
                        po[:],
                        wo_sb[:, k * D + mD * 128:k * D + (mD + 1) * 128],
                        attnT[k][:, t * 512:(t + 1) * 512],
                        start=(k == 0), stop=(k == QM - 1))
                _osb_out(t, mD, po)
            return emit

        # split oproj for the tail: accumulate k=0..2 (pairs already normed)
        # while the last pair's normalize chain runs, finish k=3 after.
        po_pre = {}

        def oproj_pre(t, mD):
            def emit():
                tag = "pj" if mD < 2 else "s2"
                po = pp.tile([128, 512], F32, tag=tag, name=f"poP{t}_{mD}")
                for k in range(QM - 1):
                    nc.tensor.matmul(
                        po[:],
                        wo_sb[:, k * D + mD * 128:k * D + (mD + 1) * 128],
                        attnT[k][:, t * 512:(t + 1) * 512],
                        start=(k == 0), stop=False)
                po_pre[(t, mD)] = po
            return emit

        def oproj_fin(t, mD):
            def emit():
                po = po_pre.pop((t, mD))
                k = QM - 1
                nc.tensor.matmul(
                    po[:],
                    wo_sb[:, k * D + mD * 128:k * D + (mD + 1) * 128],
                    attnT[k][:, t * 512:(t + 1) * 512],
                    start=False, stop=True)
                _osb_out(t, mD, po)
            return emit

        # ---------------- attention steps for one t-block ----------------
        def a_steps(t):
            pairs = []
            nu = 4 * t + 4
            for mp in range(QM):
                steps = []
                pairs.append(steps)
                st = {}
                pa = [None, None]

                def mk_pa(mp=mp, pa=pa):
                    def f():
                        pa[0] = pp.tile([65, 512], F32, tag="pa",
                                        name=f"pa0_{t}_{mp}")
                        pa[1] = pp.tile([65, 512], F32, tag="pa",
                                        name=f"pa1_{t}_{mp}")
                    return f

                def mk_S(u, mp=mp, st=st):
                    def f():
                        w = u - 4 * t
                        c0 = 0 if w < 0 else 128 * w
                        qr = qr_tiles[(t, mp)]
                        s2 = pp.tile([128, 1024], F32, tag="s2",
                                     name=f"s2_{t}_{mp}_{u}")
                        nc.tensor.matmul(s2[:, c0:512],
                                         ZkT[:, u * 128:(u + 1) * 128],
                                         qr[:, c0:512], start=True, stop=True)
                        nc.tensor.matmul(s2[:, 512 + c0:1024],
                                         ZkB[:, u * 128:(u + 1) * 128],
                                         qr[:, c0:512], start=True, stop=True)
                        st[u] = s2
                    return f

                def mk_EP(u, mp=mp, st=st, pa=pa):
                    def f():
                        w = u - 4 * t
                        c0 = 0 if w < 0 else 128 * w
                        s2 = st.pop(u)
                        pr2 = wp.tile([128, 1024], DT, tag="pr", bufs=4,
                                      name=f"pr{t}_{mp}_{u}")
                        if c0 == 0:
                            nc.scalar.activation(pr2[:], s2[:], AF.Exp,
                                                 bias=lm_sb[:, u:u + 1],
                                                 scale=SCALE)
                        else:
                            view = lambda x: x[:].rearrange(
                                "p (h q) -> p h q", h=2)[:, :, c0:512]
                            nc.scalar.activation(view(pr2), view(s2), AF.Exp,
                                                 bias=lm_sb[:, u:u + 1],
                                                 scale=SCALE)
                        if w >= 0:
                            for hh in range(2):
                                base = hh * 512 + c0
                                nc.vector.tensor_mul(pr2[:, base:base + 128],
                                                     pr2[:, base:base + 128],
                                                     tri01[:])
                        for hh in range(2):
                            nc.tensor.matmul(
                                pa[hh][:, c0:512],
                                vext[u][:, hh * 65:(hh + 1) * 65],
                                pr2[:, hh * 512 + c0:(hh + 1) * 512],
                                start=(u == 0), stop=(u == nu - 1))
                    return f

                def mk_norm(mp=mp, pa=pa):
                    def f():
                        # normalize: denominator row hops to partition 0 via
                        # a DVE shifted copy (gpsimd/custom-DVE ops mishandle
                        # partition-offset views on HW, plain DVE copies
                        # don't), then recip -> broadcast -> scale.
                        tail = (t == T - 1 and mp == QM - 1)
                        for hh in range(2):
                            a_sb = wp.tile([65, 512], F32, tag="asb", bufs=4,
                                           name=f"a{hh}_{t}_{mp}")
                            if tail:
                                nc.scalar.copy(a_sb[:], pa[hh][:])
                            else:
                                nc.vector.tensor_copy(a_sb[:], pa[hh][:])
                            rcp = wp.tile([1, 512], F32, tag="rcp", bufs=4,
                                          name=f"rcp{hh}_{t}_{mp}")
                            nc.vector.tensor_copy(rcp[0:1, :],
                                                  a_sb[64:65, :])
                            nc.vector.reciprocal_approx_fast(
                                rcp[0:1, :], rcp[0:1, :])
                            if tail:
                                # PE outer-product broadcast keeps the array
                                # busy (HAM warm) through the tail chain
                                bb = pp.tile([64, 512], F32, tag="pa",
                                             name=f"bbp{hh}_{t}_{mp}")
                                nc.tensor.matmul(bb[:], ones64[0:1, :],
                                                 rcp[0:1, :],
                                                 start=True, stop=True)
                            else:
                                bb = wp.tile([64, 512], F32, tag="bb")
                                nc.gpsimd.partition_broadcast(bb[:],
                                                              rcp[0:1, :])
                            nc.vector.tensor_mul(
                                attnT[mp][hh * 64:hh * 64 + 64,
                                          t * 512:(t + 1) * 512],
                                a_sb[0:64, :], bb[:])
                    return f

                def fuse(*fs):
                    def f():
                        for g in fs:
                            g()
                    return f

                steps.append(fuse(mk_pa(), mk_S(0)))
                steps.append(mk_S(1))
                for u in range(2, nu):
                    steps.append(fuse(mk_EP(u - 2), mk_S(u)))
                steps.append(mk_EP(nu - 2))
                if t == T - 1 and mp == QM - 1:
                    steps.append(fuse(mk_EP(nu - 1),
                                      *[oproj_pre(t, i) for i in range(4)],
                                      mk_norm()))
                else:
                    steps.append(fuse(mk_EP(nu - 1), mk_norm()))
            return pairs

        # ---------------- windowed schedule ----------------
        def run_window(t, pre=()):
            fillers = deque()
            # build filler list: chunk t+1 projections and oproj of t-1,
            # interleaved so pj-tag allocations alternate users
            proj = list(pre) + (chunk_units(t + 1) if t + 1 < T else [])
            opro = [unit_oproj(t - 1, mD) for mD in range(16)] if t >= 1 else []
            # merge proportionally
            na, nb = len(proj), len(opro)
            ia = ib = 0
            while ia < na or ib < nb:
                if ib * max(na, 1) <= ia * max(nb, 1) and ib < nb:
                    fillers.append(opro[ib]); ib += 1
                elif ia < na:
                    fillers.append(proj[ia]); ia += 1
                else:
                    fillers.append(opro[ib]); ib += 1
            pairs = a_steps(t)
            if t + 2 < T:
                load_hs(t + 2)
            flat = [s for pr in pairs for s in pr]
            if t == T - 1:
                # drain fillers before the final fused step so its oproj
                # prologue owns the pj PSUM slots (no pj alloc may sit
                # between the prologue and its finish)
                head = flat[:-1]
                tail_steps = flat[-1:]
            else:
                head = flat
                tail_steps = []
            ratio = len(fillers) / max(len(head), 1)
            acc = 0.0
            for stp in head:
                stp()
                acc += ratio
                while acc >= 1.0 and fillers:
                    fillers.popleft()()
                    acc -= 1.0
            while fillers:
                fillers.popleft()()
            for stp in tail_steps:
                stp()

        # lead-in: only the chunk-0 units A(0)'s first head-pair needs;
        # the remaining q-projections and the hs1/wo loads become window-0
        # fillers so attention matmuls start while lead-in DMAs drain.
        unit_k(0)()
        unit_q(0, 0)()
        unit_v(0)()
        pre0 = ([lambda: load_hs(1)]
                + [unit_q(0, m) for m in range(1, QM)]
                + [lambda: nc.scalar.dma_start(wo_sb[:], wot[:, :])])
        run_window(0, pre=pre0)
        for t in range(1, T):
            run_window(t)
        for mD in range(4):
            oproj_fin(T - 1, mD)()
        for mD in range(4, 16):
            unit_oproj(T - 1, mD)()
        import os
        if os.environ.get("DBG_DUMP"):
            dzt = nc.dram_tensor("dbg_zkt", [128, S], DT, kind="ExternalOutput")
            dzb = nc.dram_tensor("dbg_zkb", [128, S], DT, kind="ExternalOutput")
            nc.sync.dma_start(dzt[:, :], ZkT[:])
            nc.sync.dma_start(dzb[:, :], ZkB[:])
            dve = nc.dram_tensor("dbg_vext", [128, KT * 130], DT,
                                 kind="ExternalOutput")
            for u in range(KT):
                nc.sync.dma_start(dve[:, u * 130:(u + 1) * 130], vext[u][:])
            dat = nc.dram_tensor("dbg_attnT", [128, QM * S], DT,
                                 kind="ExternalOutput")
            for k in range(QM):
                nc.sync.dma_start(dat[:, k * S:(k + 1) * S], attnT[k][:])
        ctx.close()
    return nc


def _host_prep(hidden_states, attention_mask, Wq, Wk, Wv, Wo):
    bf16 = ml_dtypes.bfloat16
    hs = np.asarray(hidden_states, np.float32)
    am = np.asarray(attention_mask)
    Wq = np.asarray(Wq, np.float32)
    Wk = np.asarray(Wk, np.float32)
    Wv = np.asarray(Wv, np.float32)
    Wo = np.asarray(Wo, np.float32)

    # rope tables in the 16-interleaved head-dim layout
    inv = 1.0 / (ROPE_BASE ** (np.arange(0, HD, 2, dtype=np.float64) / HD))
    freqs = np.arange(S, dtype=np.float64)[:, None] * inv[None, :]  # [S, 32]
    cosf = np.cos(freqs)
    sinf = np.sin(freqs)
    fidx = np.array([r % 16 + 16 * (r // 32) for r in range(64)])
    sign = np.array([-1.0 if (r // 16) % 2 == 0 else 1.0 for r in range(64)])
    cos64 = cosf[:, fidx].T                      # [64, S]
    sin64 = (sinf[:, fidx] * sign[None, :]).T    # [64, S]
    cos128 = np.concatenate([cos64, cos64], 0).astype(np.float32)
    sin128 = np.concatenate([sin64, sin64], 0).astype(np.float32)

    def perm16(w):  # [D, 64] head-dim cols -> 16-interleaved e/o layout
        return np.concatenate(
            [w[:, 0:32:2], w[:, 1:32:2], w[:, 32:64:2], w[:, 33:64:2]], axis=1)

    def tile_rows(w, ncols):  # [DK*128, ncols] -> [128, DK*ncols]
        return np.ascontiguousarray(
            w.reshape(DK, 128, ncols).transpose(1, 0, 2).reshape(128, DK * ncols))

    hst_b = []
    for b in range(B):
        hb = hs[b].reshape(T, CH, DK, 128).transpose(3, 0, 2, 1)
        hst_b.append(np.ascontiguousarray(hb.reshape(128, T * DK * CH)).astype(bf16))
    lm_b = []
    for b in range(B):
        lm = np.where(am[b] > 0, 0.0, -1e30).astype(np.float32)
        lm_b.append(np.ascontiguousarray(lm.reshape(KT, 128).T))

    in_maps = []
    for core in range(N_CORES):
        b, g = core // TP, core % TP
        h0 = g * HQ
        kv0 = g * HKV
        # q: m-tile mp = [head h0+mp | head h0+mp+4], both perm16'd
        wq_c = np.concatenate(
            [np.concatenate([perm16(Wq[:, (h0 + mp) * HD:(h0 + mp + 1) * HD]),
                             perm16(Wq[:, (h0 + mp + 4) * HD:(h0 + mp + 5) * HD])],
                            axis=1)
             for mp in range(QM)], axis=1)                      # [D, 512]
        wk_c = np.concatenate(
            [perm16(Wk[:, (kv0 + j) * HD:(kv0 + j + 1) * HD]) for j in range(2)],
            axis=1)                                             # [D, 128]
        wv_c = np.concatenate(
            [Wv[:, (kv0 + j) * HD:(kv0 + j + 1) * HD] for j in range(2)], axis=1)
        wkv_c = np.concatenate([wk_c, wv_c], axis=1)            # [D, 256]
        # o: row-block k = [head h0+k | head h0+k+4]
        wo_c = np.concatenate(
            [np.concatenate([Wo[(h0 + k) * HD:(h0 + k + 1) * HD, :],
                             Wo[(h0 + k + 4) * HD:(h0 + k + 5) * HD, :]], axis=0)
             for k in range(QM)], axis=0)                       # [512, D]
        wqt = np.concatenate(
            [tile_rows(np.ascontiguousarray(wq_c[:, m * 128:(m + 1) * 128]), 128)
             for m in range(QM)], axis=1)
        wkvt = np.concatenate(
            [tile_rows(wk_c, 128), tile_rows(np.ascontiguousarray(wv_c), 128)],
            axis=1)
        in_maps.append({
            "hst": hst_b[b],
            "wqt": np.ascontiguousarray(wqt).astype(bf16),
            "wkvt": np.ascontiguousarray(wkvt).astype(bf16),
            "wot": np.ascontiguousarray(
                wo_c.reshape(QM, 128, D).transpose(1, 0, 2)
                .reshape(128, QM * D)).astype(bf16),
            "cosd": cos128.astype(bf16),
            "sind": sin128.astype(bf16),
            "lmd": lm_b[b],
        })
    return in_maps


_NC_CACHE = {}


def _get_nc():
    if "nc" not in _NC_CACHE:
        nc = bacc.Bacc("TRN2", target_bir_lowering=False, num_devices=N_CORES)
        build(nc)
        nc.compile()
        _NC_CACHE["nc"] = nc
    return _NC_CACHE["nc"]


def kernel(hidden_states, attention_mask, Wq, Wk, Wv, Wo):
    nc = _get_nc()
    in_maps = _host_prep(hidden_states, attention_mask, Wq, Wk, Wv, Wo)
    res = run_bass_kernel_spmd(nc, in_maps, list(range(N_CORES)))
    out = np.zeros((B, S, D), np.float32)
    for core, r in enumerate(res.results):
        o = r["outt"].reshape(128, 16, T, 512).transpose(1, 0, 2, 3)
        out[core // TP] += o.reshape(D, S).T
    return out
